# revision 35
# baseline (speedup 1.0000x reference)
"""Causal self-attention (B=2, L=2048, E=2048, H=16, HD=128) on 8 trn2 cores.

Sharding: core c = (b, g) with b = c // 4 (batch), g = c % 4 (head group of 4).
Each core computes QKV projection for its 4 heads on its batch, causal
attention with RoPE, and a partial output projection (its heads' slice of
w_proj rows). Host sums the 4 partial projections per batch.

All matmuls run in bf16/fp16 with fp32 PSUM accumulation.

Key device-side structure (per core):
  - phase 1, per 512-wide l-chunk: q/k/v projections as K-accumulated
    matmuls; rope fused right behind each q/k chunk entirely on the DVE:
        rot = (q * cs) - qshuffle(q * ss)
    where qshuffle is a 32-partition-quadrant half swap (STREAM_SHUFFLE);
    the head-dim rows are permuted host-side so each rope pair partner
    lives in the same quadrant.  cs/ss are host-prebuilt [128, L] tables
    (softmax scale folded in).
  - phase 2: scores computed transposed (sT[j,i] = k_j . q_i) so P@V needs
    no transpose; softmax without max-subtraction; exp emitted in fp16
    scaled by 2^-10 (activation bias = -ln 1024) so probabilities can be
    block-accumulated on the DVE; the softmax denominator is ONE ones-matmul
    per (head, l-chunk) on the accumulated tile (also broadcasts Z across
    partitions); causal masking by skipping upper-triangle blocks + 4
    static diagonal masks; software-pipelined with a 3-deep score-matmul
    lookahead.
  - phase 3: partial out-projection in [l, f] orientation (lhsT = y tile,
    rhs = w_proj rows) emitted per l-chunk as soon as its 4 heads finish;
    stores are full-E rows ([128, 2048] fp16, 4 KB per partition line).

Device layouts (per core):
  xt    [E=2048, L=2048] bf16   x[b].T  (e on rows); loaded as [128, 1024]
                                tiles (2 KB DMA lines)
  wqk   [E, 1024]        bf16   8 col-blocks: q-heads 0..3, k-heads 0..3,
                                head rows perm'd quadrant-pairwise, transposed
  wv    [E, 512]         bf16   v weights, natural order, transposed
  wout  [512, E]         bf16   w_proj[:, g*512:(g+1)*512].T
  cs,ss [128, L]         bf16   rope cos / (-sin|+sin per quadrant) tables
                                * 128**-0.25
  masks [128, 4*512]     fp16   causal diagonal-block masks
  ones  [128, 128]       fp16   all-ones (softmax denominator broadcast-sum)
Output:
  out   [L, E] fp16  (partial projection; host adds in fp32)
"""

from contextlib import ExitStack

import numpy as np
import ml_dtypes

import concourse.bass as bass
import concourse.mybir as mybir
import concourse.tile as tile
from concourse import bacc
from concourse.bass_utils import run_bass_kernel_spmd

BF16 = ml_dtypes.bfloat16
B, L, E, H, HD = 2, 2048, 2048, 16, 128
G = 4            # head groups (cores per batch)
HPG = H // G     # heads per group = 4
NCORES = 8
NE = E // 128    # 16 e-chunks
NLC = L // 512   # 4 l-chunks of 512
NLT = L // 128   # 16 l-tiles of 128
SCALE = float(128.0 ** -0.25)   # per-operand score scale (q and k each)
EXPBIAS = -6.931471805599453    # -ln(1024): p scaled 2^-10, cancels in y=psy/Z

FP32 = mybir.dt.float32
FP16 = mybir.dt.float16
BF = mybir.dt.bfloat16

# rope partner swap within each 32-partition quadrant (i <-> i+16)
SWAP_MASK = list(range(16, 32)) + list(range(0, 16))


def build_nc():
    nc = bacc.Bacc(
        "TRN2",
        target_bir_lowering=False,
        debug=False,
        enable_asserts=False,
        num_devices=NCORES,
    )
    d = {}
    d["xt"] = nc.dram_tensor("xt", [E, L], BF, kind="ExternalInput").ap()
    d["wqk"] = nc.dram_tensor("wqk", [E, 2 * HPG * 128], BF, kind="ExternalInput").ap()
    d["wv"] = nc.dram_tensor("wv", [E, HPG * 128], BF, kind="ExternalInput").ap()
    d["wout"] = nc.dram_tensor("wout", [HPG * 128, E], BF, kind="ExternalInput").ap()
    d["cs"] = nc.dram_tensor("cs", [128, L], BF, kind="ExternalInput").ap()
    d["ss"] = nc.dram_tensor("ss", [128, L], BF, kind="ExternalInput").ap()
    d["masks"] = nc.dram_tensor("masks", [128, 4 * 512], FP16,
                                kind="ExternalInput").ap()
    d["ones"] = nc.dram_tensor("ones", [128, 128], FP16, kind="ExternalInput").ap()
    d["out"] = nc.dram_tensor("out", [L, E], mybir.dt.float16,
                              kind="ExternalOutput").ap()

    with tile.TileContext(nc) as tc:
        build_kernel(tc, d)
    nc.compile()
    return nc


def build_kernel(tc, d):
    nc = tc.nc
    EXP = mybir.ActivationFunctionType.Exp

    with ExitStack() as ctx:
        const = ctx.enter_context(tc.tile_pool(name="const", bufs=1))
        qkres = ctx.enter_context(tc.tile_pool(name="qkres", bufs=1))
        vres = ctx.enter_context(tc.tile_pool(name="vres", bufs=1))
        yres = ctx.enter_context(tc.tile_pool(name="yres", bufs=1))

        # ---- resident constants / weights ----
        cs_sb = const.tile([128, L], BF, name="cs_sb", tag="cs_sb")
        ss_sb = const.tile([128, L], BF, name="ss_sb", tag="ss_sb")
        wv_sb = const.tile([128, NE, HPG * 128], BF, name="wv_sb", tag="wv_sb")
        wv_r = d["wv"].rearrange("(ec p) f -> p ec f", p=128)
        # masks/ones/wout are needed only from the attention phase on; their
        # DMAs are emitted after phase 1 so they don't contend with the
        # startup-critical weight/x loads. Tiles allocated here.
        masks_sb = const.tile([128, 4, 512], FP16, name="masks_sb", tag="masks_sb")
        ones_sb = const.tile([128, 128], FP16, name="ones_sb", tag="ones_sb")
        wout_sb = const.tile([128, HPG, E], BF, name="wout_sb", tag="wout_sb")
        ebias_sb = const.tile([128, 1], FP32, name="ebias_sb", tag="ebias_sb")
        nc.vector.memset(ebias_sb, EXPBIAS)

        # ---- residents ----
        q_sb = [qkres.tile([128, L], BF, name=f"q_sb{h}", tag=f"q_sb{h}")
                for h in range(HPG)]
        k_sb = [qkres.tile([128, L], BF, name=f"k_sb{h}", tag=f"k_sb{h}")
                for h in range(HPG)]
        v_sb = vres.tile([128, NLT, HPG * 128], FP16, name="v_sb", tag="v_sb")
        y_sb = [yres.tile([128, L], BF, name=f"y_sb{h}", tag=f"y_sb{h}")
                for h in range(HPG)]

        # ================= phase 1: QKV projection + fused rope ============
        # phase-1-only pools (wqk weights, x tiles, rope temporaries): closed
        # after phase 1 so their SBUF is reusable by the attention pools.
        with tc.tile_pool(name="wqks", bufs=1) as wqks, \
             tc.tile_pool(name="xs", bufs=24) as xs, \
             tc.tile_pool(name="atile", bufs=12) as atile, \
             tc.tile_pool(name="psum1", bufs=1, space="PSUM") as ps1:

            wqk_sb = wqks.tile([128, NE, 2 * HPG * 128], BF, name="wqk_sb",
                               tag="wqk_sb")
            wqk_r = d["wqk"].rearrange("(ec p) f -> p ec f", p=128)
            # All wqk on the gpsimd queue: first chunks as singles (earliest
            # arrival), the rest as pairs (amortizes the ~0.5us per-dma ring
            # overhead so delivery keeps up with the PE's ~1.7us/chunk
            # consumption).  x tiles split across sync+scalar (see below) --
            # aggregate HBM bandwidth is the startup constraint.
            for e in range(4):
                nc.gpsimd.dma_start(out=wqk_sb[:, e, :], in_=wqk_r[:, e, :])
            for e in range(4, NE, 2):
                nc.gpsimd.dma_start(out=wqk_sb[:, e:e + 2, :],
                                    in_=wqk_r[:, e:e + 2, :])

            def acc_tile(nm):
                return ps1.tile([128, 512], FP32, name=nm, tag="pacc", bufs=8)

            # PE warmup: dummy matmuls on never-written SBUF fill the idle
            # window while the first weights stream in, so the tensor engine's
            # clock ramp completes before real work starts (results unused)
            dummy_sb = wqks.tile([128, 512], BF, name="dummy_sb", tag="dummy")
            nc.vector.memset(dummy_sb, 0)
            for i in range(8):
                wt = acc_tile(f"warm{i}")
                nc.tensor.matmul(wt, lhsT=dummy_sb[:, :128], rhs=dummy_sb,
                                 start=True, stop=True)

            xt_pair = []  # [128, 1024] tiles covering l-chunks (2p, 2p+1)

            for lc in range(NLC):
                ls_lo = lc * 512
                cs_lc = cs_sb[:, ls_lo:ls_lo + 512]
                ss_lc = ss_sb[:, ls_lo:ls_lo + 512]

                if lc % 2 == 0:
                    # x tiles loaded 1024 wide (2 KB per partition line).
                    # Startup (pair 0): even e on sync, odd e on scalar so the
                    # two streams keep the PE's e-cadence.  Pair 1 is not
                    # needed until ~100us, so it all goes on scalar BEHIND wv,
                    # clearing the startup bandwidth crunch.
                    xt_pair = []
                    for e in range(NE):
                        t = xs.tile([128, 1024], BF, name=f"xt_{lc}_{e}",
                                    tag="xt")
                        if lc == 0:
                            eng = nc.sync if e % 2 == 0 else nc.scalar
                        else:
                            eng = nc.scalar
                        eng.dma_start(
                            out=t,
                            in_=d["xt"][e * 128:(e + 1) * 128,
                                        ls_lo:ls_lo + 1024])
                        xt_pair.append(t)
                    if lc == 0:
                        # rope tables / v weights ride behind the first x
                        # half-streams on their respective queues
                        nc.sync.dma_start(out=cs_sb, in_=d["cs"])
                        nc.sync.dma_start(out=ss_sb, in_=d["ss"])
                        for e in range(0, NE, 2):
                            nc.scalar.dma_start(out=wv_sb[:, e:e + 2, :],
                                                in_=wv_r[:, e:e + 2, :])
                sub = (lc % 2) * 512

                def xsl(e, a, b):
                    # slice of this l-chunk's half of the [128, 1024] x tile
                    return xt_pair[e][:, sub + a:sub + b]

                def qk_pass(halves):
                    # projection matmuls for the given f-block halves
                    # (0 = q heads, 1 = k heads); passing both interleaves
                    # them per e-chunk, which halves the weight-chunk arrival
                    # rate the PE needs (used for the DMA-bound first l-chunk)
                    acc = [acc_tile(f"p{half}_{lc}_{h}")
                           for half in halves for h in range(HPG)]
                    for e in range(NE):
                        for i, half in enumerate(halves):
                            for h in range(HPG):
                                fb = half * HPG + h
                                nc.tensor.matmul(
                                    acc[i * HPG + h],
                                    lhsT=wqk_sb[:, e, fb * 128:(fb + 1) * 128],
                                    rhs=xsl(e, 0, 512),
                                    start=(e == 0), stop=(e == NE - 1))
                    return acc

                def rope_a(acc, which):
                    # a = q*ss (bf16), dst-slice = q*cs ; releases acc
                    a_t = []
                    for h in range(HPG):
                        a = atile.tile([128, 512], BF,
                                       name=f"a_{which}{h}_{lc}", tag="a")
                        nc.vector.tensor_mul(out=a, in0=acc[h], in1=ss_lc)
                        dst = (q_sb if which == "q" else k_sb)[h]
                        nc.vector.tensor_mul(
                            out=dst[:, ls_lo:ls_lo + 512], in0=acc[h], in1=cs_lc)
                        a_t.append(a)
                    return a_t

                def rope_b(a_t, which):
                    # dst -= quadrant_swap(a)   (all on DVE)
                    for h in range(HPG):
                        ash = atile.tile([128, 512], BF,
                                         name=f"ash_{which}{h}_{lc}", tag="a")
                        nc.vector.stream_shuffle(out=ash, in_=a_t[h],
                                                 mask=SWAP_MASK)
                        dst = (q_sb if which == "q" else k_sb)[h]
                        sl = dst[:, ls_lo:ls_lo + 512]
                        nc.vector.tensor_sub(out=sl, in0=sl, in1=ash)

                def v_pass():
                    # v pass (x tiles stationary -> natural [l, d] layout);
                    # e-outer so each wv chunk is consumed once, in its DMA
                    # arrival order
                    accv = [acc_tile(f"pv_{lc}_{ls}") for ls in range(4)]
                    for e in range(NE):
                        for ls in range(4):
                            nc.tensor.matmul(
                                accv[ls],
                                lhsT=xsl(e, ls * 128, (ls + 1) * 128),
                                rhs=wv_sb[:, e, :],
                                start=(e == 0), stop=(e == NE - 1))
                    for ls in range(4):
                        if ls % 2 == 0:
                            nc.scalar.copy(out=v_sb[:, lc * 4 + ls, :],
                                           in_=accv[ls])
                        else:
                            nc.vector.tensor_copy(out=v_sb[:, lc * 4 + ls, :],
                                                  in_=accv[ls])

                if lc == 0:
                    acc8 = qk_pass((0, 1))
                    accq, acck = acc8[:HPG], acc8[HPG:]
                    aq = rope_a(accq, "q")
                    ak = rope_a(acck, "k")
                    rope_b(aq, "q")
                    v_pass()
                    rope_b(ak, "k")
                elif lc == NLC - 1:
                    # last chunk: finalize k before the v pass so attention's
                    # first score matmuls aren't gated on the v matmuls
                    accq = qk_pass((0,))
                    aq = rope_a(accq, "q")
                    acck = qk_pass((1,))
                    rope_b(aq, "q")
                    ak = rope_a(acck, "k")
                    rope_b(ak, "k")
                    v_pass()
                else:
                    accq = qk_pass((0,))
                    aq = rope_a(accq, "q")
                    acck = qk_pass((1,))
                    rope_b(aq, "q")
                    ak = rope_a(acck, "k")
                    v_pass()
                    rope_b(ak, "k")

        # ======== phase 2+3: causal attention with interleaved projection ==
        # jobs are ic-major: once all 4 heads finished l-chunk ic, that
        # chunk's output projection is emitted immediately — it fills
        # attention pipeline bubbles and spreads the output DMA.
        nc.gpsimd.dma_start(
            out=masks_sb, in_=d["masks"].rearrange("p (r f) -> p r f", r=4))
        nc.gpsimd.dma_start(out=ones_sb, in_=d["ones"])
        with tc.tile_pool(name="pexp", bufs=10) as pexp, \
             tc.tile_pool(name="zacc", bufs=3) as zacc, \
             tc.tile_pool(name="zpool", bufs=3) as zpool, \
             tc.tile_pool(name="outst", bufs=3) as outst, \
             tc.tile_pool(name="psum2", bufs=1, space="PSUM") as ps2:
            jobs = [(h, ic) for ic in range(NLC) for h in range(HPG)]
            # per-job block order: diagonal blocks first (their DVE mask-muls
            # land while the za chain is short), then the full blocks -- the
            # job tail is then pure adds and the DVE catches up before the
            # denominator matmuls.  (pos, jb): pos is the processing index.
            jseq = {ji: list(range(4 * ic, 4 * ic + 4)) + list(range(4 * ic))
                    for ji, (_h, ic) in enumerate(jobs)}
            steps = [(ji, pos)
                     for ji in range(len(jobs))
                     for pos in range(len(jseq[ji]))]
            LA = 2
            pss_map = {}
            psy_map = {}
            za_map = {}
            psz_map = {}
            # output-projection work queue: quarters (lcx, lt, fq) are
            # interleaved one-per-step into the FOLLOWING chunk's attention
            # jobs -- the projection is scalar-light and PE-heavy, which pads
            # each window so the scalar engine's exp stream keeps up
            pending_proj = []
            ot_map = {}
            proj_stride = [1]
            proj_tick = [0]

            def emit_proj_quarter():
                lcx, lt, fq = pending_proj.pop(0)
                l0 = lcx * 512 + lt * 128
                if fq == 0:
                    ot_map[(lcx, lt)] = outst.tile(
                        [128, E], mybir.dt.float16,
                        name=f"ot_{lcx}_{lt}", tag="ot")
                ot = ot_map[(lcx, lt)]
                po = ps2.tile([128, 512], FP32, name=f"po_{lcx}_{lt}_{fq}",
                              tag="po", bufs=2)
                for hh in range(HPG):
                    nc.tensor.matmul(
                        po,
                        lhsT=y_sb[hh][:, l0:l0 + 128],
                        rhs=wout_sb[:, hh, fq * 512:(fq + 1) * 512],
                        start=(hh == 0), stop=(hh == HPG - 1))
                if fq < 3:
                    nc.vector.tensor_copy(
                        out=ot[:, fq * 512:(fq + 1) * 512], in_=po)
                else:
                    nc.scalar.copy(
                        out=ot[:, fq * 512:(fq + 1) * 512], in_=po)
                    eng = (nc.sync, nc.gpsimd, nc.scalar, nc.sync)[lt % 4]
                    eng.dma_start(out=d["out"][l0:l0 + 128, :],
                                  in_=ot_map.pop((lcx, lt)))

            def emit_s(ji, pos):
                h, ic = jobs[ji]
                jb = jseq[ji][pos]
                # diagonal blocks (r >= 1) have no valid columns below
                # f = 128*r: compute only the valid column range
                r = jb - 4 * ic
                lo = r * 128 if r > 0 else 0
                t = ps2.tile([128, 512], FP32, name=f"pss_{ji}_{jb}",
                             tag="pss", bufs=3)
                nc.tensor.matmul(
                    t[:, lo:],
                    lhsT=k_sb[h][:, jb * 128:(jb + 1) * 128],
                    rhs=q_sb[h][:, ic * 512 + lo:(ic + 1) * 512],
                    start=True, stop=True)
                pss_map[(ji, jb)] = t

            ptr = 0
            for idx, (ji, pos) in enumerate(steps):
                while ptr < len(steps) and ptr <= idx + LA:
                    emit_s(*steps[ptr])
                    ptr += 1
                h, ic = jobs[ji]
                njb = 4 * ic + 4
                jb = jseq[ji][pos]
                if ji == 1 and pos == 0:
                    # wout is first needed by the ic=0 projection (~30us into
                    # the attention phase): deferring its load here keeps it
                    # clear of the startup bandwidth crunch
                    nc.gpsimd.dma_start(
                        out=wout_sb,
                        in_=d["wout"].rearrange("(h p) f -> p h f", p=128))
                if pos == 0:
                    psy_map[ji] = ps2.tile([128, 512], FP32, name=f"psy_{ji}",
                                           tag="psy", bufs=2)
                psy = psy_map[ji]
                pss = pss_map.pop((ji, jb))
                r = jb - 4 * ic
                lo = r * 128 if r > 0 else 0
                pt = pexp.tile([128, 512], FP16, name=f"pt_{ji}_{jb}",
                               tag="pexp")
                nc.scalar.activation(out=pt[:, lo:], in_=pss[:, lo:],
                                     func=EXP, bias=ebias_sb)
                if r >= 0:
                    # diagonal block: only the first 128 columns of the valid
                    # range hold the per-element triangle; the rest are all-1.
                    # On gpsimd -- the DVE is loaded with the za chain.
                    nc.gpsimd.tensor_mul(
                        out=pt[:, lo:lo + 128], in0=pt[:, lo:lo + 128],
                        in1=masks_sb[:, r, lo:lo + 128])
                # running probability sum for the softmax denominator (DVE);
                # the final block is NOT accumulated -- it goes straight into
                # the second denominator matmul, so the PE never waits on the
                # full exp->add chain at the job end
                if pos == 0:
                    za = zacc.tile([128, 512], FP16, name=f"za_{ji}", tag="za")
                    za_map[ji] = za
                    nc.vector.tensor_copy(out=za, in_=pt)
                elif pos < njb - 1:
                    za = za_map[ji]
                    nc.vector.tensor_add(out=za[:, lo:], in0=za[:, lo:],
                                         in1=pt[:, lo:])
                if pos == njb - 2:
                    # first njb-1 blocks accumulated: emit the first
                    # denominator matmul now (also broadcasts Z across
                    # partitions)
                    psz = ps2.tile([128, 512], FP32, name=f"psz_{ji}",
                                   tag="psz", bufs=1)
                    psz_map[ji] = psz
                    nc.tensor.matmul(psz, lhsT=ones_sb, rhs=za_map.pop(ji),
                                     start=True, stop=False)
                nc.tensor.matmul(psy[:, lo:],
                                 lhsT=v_sb[:, jb, h * 128:(h + 1) * 128],
                                 rhs=pt[:, lo:],
                                 start=(pos == 0), stop=(pos == njb - 1))
                if pos == njb - 1:
                    # fold the final block's probabilities into Z directly
                    psz = psz_map.pop(ji)
                    nc.tensor.matmul(psz[:, lo:], lhsT=ones_sb,
                                     rhs=pt[:, lo:], start=False, stop=True)
                    zv = zpool.tile([128, 512], FP32, name=f"zinv_{ji}",
                                    tag="zinv")
                    nc.vector.reciprocal_approx_fast(out=zv, in_=psz)
                    nc.vector.tensor_mul(
                        out=y_sb[h][:, ic * 512:(ic + 1) * 512],
                        in0=psy_map.pop(ji), in1=zv)
                    if h == HPG - 1:
                        # all heads done for this l-chunk: queue its output
                        # projection ([l, f] orientation, full-E rows so the
                        # store DMA moves 4 KB per partition line); quarters
                        # are drained one per following step
                        pending_proj.extend(
                            (ic, lt, fq) for lt in range(4) for fq in range(4))
                        # spread the 16 quarters evenly over the next chunk's
                        # steps so every following job gets PE padding
                        if ic + 1 < NLC:
                            nsteps = HPG * (4 * (ic + 1) + 4)
                            proj_stride[0] = max(1, nsteps // 16)
                        else:
                            proj_stride[0] = 1
                        proj_tick[0] = 0
                if pending_proj:
                    proj_tick[0] += 1
                    if proj_tick[0] % proj_stride[0] == 0:
                        emit_proj_quarter()

            # final chunk's projection has no following steps: drain it
            while pending_proj:
                emit_proj_quarter()


# ------------------------------------------------------------------ host side

# head-dim permutation: quadrant q holds rope pairs 16q..16q+15 as
# (even dims | odd dims), so the rope partner swap stays within a
# 32-partition quadrant (STREAM_SHUFFLE's reach)
_PERM_IDX = np.concatenate(
    [np.concatenate([np.arange(16) * 2 + 32 * q,
                     np.arange(16) * 2 + 1 + 32 * q])
     for q in range(4)])


def prep_in_maps(x, rope, w_attn, w_proj):
    x = np.asarray(x, np.float32)
    rope = np.asarray(rope, np.float32)
    w_attn = np.asarray(w_attn, np.float32)
    w_proj = np.asarray(w_proj, np.float32)

    sin = rope[:, :, 0].T                    # [64, L]
    cos = rope[:, :, 1].T
    cs = np.zeros((128, L), np.float32)
    ss = np.zeros((128, L), np.float32)
    for q in range(4):
        pr = slice(16 * q, 16 * (q + 1))     # pair indices of quadrant q
        cs[32 * q:32 * q + 16] = cos[pr]
        cs[32 * q + 16:32 * q + 32] = cos[pr]
        ss[32 * q:32 * q + 16] = -sin[pr]
        ss[32 * q + 16:32 * q + 32] = sin[pr]
    cs = (cs * SCALE).astype(BF16)
    ss = (ss * SCALE).astype(BF16)

    p = np.arange(128)[:, None]
    f = np.arange(512)[None, :]
    masks = np.zeros((128, 4, 512), np.float32)
    for r in range(4):
        masks[:, r, :] = (r * 128 + p <= f).astype(np.float32)
    masks = masks.reshape(128, 4 * 512).astype(np.float16)

    ones = np.ones((128, 128), np.float16)

    xt_b = [np.ascontiguousarray(x[b].T).astype(BF16) for b in range(B)]

    wqk_g, wv_g, wout_g = {}, {}, {}
    for g in range(G):
        heads = [g * HPG + hl for hl in range(HPG)]
        wq = [np.ascontiguousarray(
                 w_attn[h * 128:(h + 1) * 128, :][_PERM_IDX, :].T) for h in heads]
        wk = [np.ascontiguousarray(
                 w_attn[E + h * 128:E + (h + 1) * 128, :][_PERM_IDX, :].T)
              for h in heads]
        wqk_g[g] = np.concatenate(wq + wk, axis=1).astype(BF16)        # [E, 1024]
        wv_g[g] = np.concatenate(
            [w_attn[2 * E + h * 128:2 * E + (h + 1) * 128, :].T for h in heads],
            axis=1).astype(BF16)                                        # [E, 512]
        wout_g[g] = np.ascontiguousarray(
            w_proj[:, g * 512:(g + 1) * 512].T).astype(BF16)            # [512, E]

    in_maps = []
    for c in range(NCORES):
        b, g = divmod(c, G)
        in_maps.append({
            "xt": xt_b[b],
            "wqk": wqk_g[g],
            "wv": wv_g[g],
            "wout": wout_g[g],
            "cs": cs,
            "ss": ss,
            "masks": masks,
            "ones": ones,
        })
    return in_maps


def assemble_output(results):
    out = np.zeros((B, L, E), np.float32)
    for c in range(NCORES):
        b, g = divmod(c, G)
        out[b] += results[c]["out"]
    return out


_NC = None


def get_nc():
    global _NC
    if _NC is None:
        _NC = build_nc()
    return _NC


def run(x, rope, w_attn, w_proj, trace=False, tmpdir=None):
    nc = get_nc()
    in_maps = prep_in_maps(x, rope, w_attn, w_proj)
    kwargs = {}
    if trace:
        import sys
        import types
        from concourse import bass_utils as _bu
        try:
            from trn_agent_boot.trn_boot import _ntff_profile_via_ctypes
            hook = _ntff_profile_via_ctypes("/opt/axon/libaxon_pjrt.so")
            mod = types.ModuleType("antenv.axon_hooks")
            mod.get_axon_ntff_profile_hook = lambda: hook
            sys.modules["antenv.axon_hooks"] = mod
            _bu.upload_artifacts = lambda dd: dd
        except Exception as e:  # pragma: no cover
            print("trace hook unavailable:", e)
        kwargs = dict(trace=True, tmpdir=tmpdir)
    res = run_bass_kernel_spmd(nc, in_maps, core_ids=list(range(NCORES)), **kwargs)
    return assemble_output(res.results), res


def kernel(x, rope, w_attn, w_proj):
    out, _ = run(x, rope, w_attn, w_proj, trace=False)
    return out


# revision 42
# speedup vs baseline: 1.1887x; 1.1887x over previous
"""Causal self-attention (B=2, L=2048, E=2048, H=16, HD=128) on 8 trn2 cores.

Sharding: core c = (b, g) with b = c // 4 (batch), g = c % 4 (head group of 4).
Each core computes QKV projection for its 4 heads on its batch, causal
attention with RoPE, and a partial output projection (its heads' slice of
w_proj rows). Host sums the 4 partial projections per batch.

All matmuls run in bf16/fp16 with fp32 PSUM accumulation.

Key device-side structure (per core):
  - phase 1, per 512-wide l-chunk: q/k/v projections as K-accumulated
    matmuls; rope fused right behind each q/k chunk entirely on the DVE:
        rot = (q * cs) - qshuffle(q * ss)
    where qshuffle is a 32-partition-quadrant half swap (STREAM_SHUFFLE);
    the head-dim rows are permuted host-side so each rope pair partner
    lives in the same quadrant.  cs/ss are host-prebuilt [128, L] tables
    (softmax scale folded in).
  - phase 2: scores computed transposed (sT[j,i] = k_j . q_i) so P@V needs
    no transpose; softmax without max-subtraction; exp emitted in fp16
    scaled by 2^-10 (activation bias = -ln 1024) so probabilities can be
    block-accumulated on the DVE; the softmax denominator is ONE ones-matmul
    per (head, l-chunk) on the accumulated tile (also broadcasts Z across
    partitions); causal masking by skipping upper-triangle blocks + 4
    static diagonal masks; software-pipelined with a 3-deep score-matmul
    lookahead.
  - phase 3: partial out-projection in [l, f] orientation (lhsT = y tile,
    rhs = w_proj rows) emitted per l-chunk as soon as its 4 heads finish;
    stores are full-E rows ([128, 2048] fp16, 4 KB per partition line).

Device layouts (per core):
  xt    [E=2048, L=2048] bf16   x[b].T  (e on rows); loaded as [128, 1024]
                                tiles (2 KB DMA lines)
  wqk   [E, 1024]        bf16   8 col-blocks: q-heads 0..3, k-heads 0..3,
                                head rows perm'd quadrant-pairwise, transposed
  wv    [E, 512]         bf16   v weights, natural order, transposed
  wout  [512, E]         bf16   w_proj[:, g*512:(g+1)*512].T
  cs,ss [128, L]         bf16   rope cos / (-sin|+sin per quadrant) tables
                                * 128**-0.25
  masks [128, 4*512]     fp16   causal diagonal-block masks
  ones  [128, 128]       fp16   all-ones (softmax denominator broadcast-sum)
Output:
  out   [L, E] fp16  (partial projection; host adds in fp32)
"""

from contextlib import ExitStack

import numpy as np
import ml_dtypes

import concourse.bass as bass
import concourse.mybir as mybir
import concourse.tile as tile
from concourse import bacc
from concourse.bass_utils import run_bass_kernel_spmd

BF16 = ml_dtypes.bfloat16
B, L, E, H, HD = 2, 2048, 2048, 16, 128
G = 4            # head groups (cores per batch)
HPG = H // G     # heads per group = 4
NCORES = 8
NE = E // 128    # 16 e-chunks
NLC = L // 512   # 4 l-chunks of 512
NLT = L // 128   # 16 l-tiles of 128
SCALE = float(128.0 ** -0.25)   # per-operand score scale (q and k each)
EXPBIAS = -6.931471805599453    # -ln(1024): p scaled 2^-10, cancels in y=psy/Z

FP32 = mybir.dt.float32
FP16 = mybir.dt.float16
BF = mybir.dt.bfloat16

# rope partner swap within each 32-partition quadrant (i <-> i+16)
SWAP_MASK = list(range(16, 32)) + list(range(0, 16))


def build_nc():
    nc = bacc.Bacc(
        "TRN2",
        target_bir_lowering=False,
        debug=False,
        enable_asserts=False,
        num_devices=NCORES,
    )
    d = {}
    d["xt"] = nc.dram_tensor("xt", [E, L], BF, kind="ExternalInput").ap()
    d["wqk"] = nc.dram_tensor("wqk", [E, 2 * HPG * 128], BF, kind="ExternalInput").ap()
    d["wv"] = nc.dram_tensor("wv", [E, HPG * 128], BF, kind="ExternalInput").ap()
    d["wout"] = nc.dram_tensor("wout", [HPG * 128, E], BF, kind="ExternalInput").ap()
    d["cs"] = nc.dram_tensor("cs", [128, L], BF, kind="ExternalInput").ap()
    d["ss"] = nc.dram_tensor("ss", [128, L], BF, kind="ExternalInput").ap()
    d["masks"] = nc.dram_tensor("masks", [128, 4 * 512], FP16,
                                kind="ExternalInput").ap()
    d["ones"] = nc.dram_tensor("ones", [128, 128], FP16, kind="ExternalInput").ap()
    d["out"] = nc.dram_tensor("out", [L, E], mybir.dt.float16,
                              kind="ExternalOutput").ap()

    with tile.TileContext(nc) as tc:
        build_kernel(tc, d)
    nc.compile()
    return nc


def build_kernel(tc, d):
    nc = tc.nc
    EXP = mybir.ActivationFunctionType.Exp

    with ExitStack() as ctx:
        const = ctx.enter_context(tc.tile_pool(name="const", bufs=1))
        qkres = ctx.enter_context(tc.tile_pool(name="qkres", bufs=1))
        vres = ctx.enter_context(tc.tile_pool(name="vres", bufs=1))
        yres = ctx.enter_context(tc.tile_pool(name="yres", bufs=1))

        # ---- resident constants / weights ----
        cs_sb = const.tile([128, L], BF, name="cs_sb", tag="cs_sb")
        ss_sb = const.tile([128, L], BF, name="ss_sb", tag="ss_sb")
        wv_sb = const.tile([128, NE, HPG * 128], BF, name="wv_sb", tag="wv_sb")
        wv_r = d["wv"].rearrange("(ec p) f -> p ec f", p=128)
        # masks/ones/wout are needed only from the attention phase on; their
        # DMAs are emitted after phase 1 so they don't contend with the
        # startup-critical weight/x loads. Tiles allocated here.
        masks_sb = const.tile([128, 4, 512], FP16, name="masks_sb", tag="masks_sb")
        ones_sb = const.tile([128, 128], FP16, name="ones_sb", tag="ones_sb")
        wout_sb = const.tile([128, HPG, E], BF, name="wout_sb", tag="wout_sb")
        ebias_sb = const.tile([128, 1], FP32, name="ebias_sb", tag="ebias_sb")
        nc.vector.memset(ebias_sb, EXPBIAS)

        # ---- residents ----
        q_sb = [qkres.tile([128, L], BF, name=f"q_sb{h}", tag=f"q_sb{h}")
                for h in range(HPG)]
        k_sb = [qkres.tile([128, L], BF, name=f"k_sb{h}", tag=f"k_sb{h}")
                for h in range(HPG)]
        v_sb = vres.tile([128, NLT, HPG * 128], FP16, name="v_sb", tag="v_sb")
        y_sb = [yres.tile([128, L], BF, name=f"y_sb{h}", tag=f"y_sb{h}")
                for h in range(HPG)]

        # ================= phase 1: QKV projection + fused rope ============
        # phase-1-only pools (wqk weights, x tiles, rope temporaries): closed
        # after phase 1 so their SBUF is reusable by the attention pools.
        with tc.tile_pool(name="wqks", bufs=1) as wqks, \
             tc.tile_pool(name="xs", bufs=24) as xs, \
             tc.tile_pool(name="atile", bufs=12) as atile, \
             tc.tile_pool(name="psum1", bufs=1, space="PSUM") as ps1:

            wqk_sb = wqks.tile([128, NE, 2 * HPG * 128], BF, name="wqk_sb",
                               tag="wqk_sb")
            wqk_r = d["wqk"].rearrange("(ec p) f -> p ec f", p=128)
            # All wqk on the gpsimd queue: first chunks as singles (earliest
            # arrival), the rest as pairs (amortizes the ~0.5us per-dma ring
            # overhead so delivery keeps up with the PE's ~1.7us/chunk
            # consumption).  x tiles split across sync+scalar (see below) --
            # aggregate HBM bandwidth is the startup constraint.
            # chunk 0 rides at the head of the sync queue, which starts
            # earliest -- this de-jitters the first real matmul
            nc.sync.dma_start(out=wqk_sb[:, 0, :], in_=wqk_r[:, 0, :])
            for e in range(1, 4):
                nc.gpsimd.dma_start(out=wqk_sb[:, e, :], in_=wqk_r[:, e, :])
            for e in range(4, NE, 2):
                nc.gpsimd.dma_start(out=wqk_sb[:, e:e + 2, :],
                                    in_=wqk_r[:, e:e + 2, :])

            def acc_tile(nm):
                return ps1.tile([128, 512], FP32, name=nm, tag="pacc", bufs=8)

            # PE warmup: dummy matmuls on never-written SBUF fill the idle
            # window while the first weights stream in, so the tensor engine's
            # clock ramp completes before real work starts (results unused)
            dummy_sb = wqks.tile([128, 512], BF, name="dummy_sb", tag="dummy")
            nc.vector.memset(dummy_sb, 0)
            for i in range(8):
                wt = acc_tile(f"warm{i}")
                nc.tensor.matmul(wt, lhsT=dummy_sb[:, :128], rhs=dummy_sb,
                                 start=True, stop=True)

            xt_pair = []  # [128, 1024] tiles covering l-chunks (2p, 2p+1)

            for lc in range(NLC):
                ls_lo = lc * 512
                cs_lc = cs_sb[:, ls_lo:ls_lo + 512]
                ss_lc = ss_sb[:, ls_lo:ls_lo + 512]

                if lc % 2 == 0:
                    # x tiles loaded 1024 wide (2 KB per partition line).
                    # Startup (pair 0): even e on sync, odd e on scalar so the
                    # two streams keep the PE's e-cadence.  Pair 1 is not
                    # needed until ~100us, so it all goes on scalar BEHIND wv,
                    # clearing the startup bandwidth crunch.
                    xt_pair = []
                    for e in range(NE):
                        t = xs.tile([128, 1024], BF, name=f"xt_{lc}_{e}",
                                    tag="xt")
                        if lc == 0:
                            eng = nc.sync if e % 2 == 0 else nc.scalar
                        else:
                            eng = nc.scalar
                        eng.dma_start(
                            out=t,
                            in_=d["xt"][e * 128:(e + 1) * 128,
                                        ls_lo:ls_lo + 1024])
                        xt_pair.append(t)
                    if lc == 0:
                        # rope tables / v weights ride behind the first x
                        # half-streams on their respective queues
                        nc.sync.dma_start(out=cs_sb, in_=d["cs"])
                        nc.sync.dma_start(out=ss_sb, in_=d["ss"])
                        for e in range(0, NE, 2):
                            nc.scalar.dma_start(out=wv_sb[:, e:e + 2, :],
                                                in_=wv_r[:, e:e + 2, :])
                sub = (lc % 2) * 512

                def xsl(e, a, b):
                    # slice of this l-chunk's half of the [128, 1024] x tile
                    return xt_pair[e][:, sub + a:sub + b]

                def qk_pass(halves):
                    # projection matmuls for the given f-block halves
                    # (0 = q heads, 1 = k heads); passing both interleaves
                    # them per e-chunk, which halves the weight-chunk arrival
                    # rate the PE needs (used for the DMA-bound first l-chunk)
                    acc = [acc_tile(f"p{half}_{lc}_{h}")
                           for half in halves for h in range(HPG)]
                    for e in range(NE):
                        for i, half in enumerate(halves):
                            for h in range(HPG):
                                fb = half * HPG + h
                                nc.tensor.matmul(
                                    acc[i * HPG + h],
                                    lhsT=wqk_sb[:, e, fb * 128:(fb + 1) * 128],
                                    rhs=xsl(e, 0, 512),
                                    start=(e == 0), stop=(e == NE - 1))
                    return acc

                def rope_a(acc, which):
                    # a = q*ss (bf16), dst-slice = q*cs ; releases acc
                    a_t = []
                    for h in range(HPG):
                        a = atile.tile([128, 512], BF,
                                       name=f"a_{which}{h}_{lc}", tag="a")
                        nc.vector.tensor_mul(out=a, in0=acc[h], in1=ss_lc)
                        dst = (q_sb if which == "q" else k_sb)[h]
                        nc.vector.tensor_mul(
                            out=dst[:, ls_lo:ls_lo + 512], in0=acc[h], in1=cs_lc)
                        a_t.append(a)
                    return a_t

                def rope_b(a_t, which):
                    # dst -= quadrant_swap(a)   (all on DVE)
                    for h in range(HPG):
                        ash = atile.tile([128, 512], BF,
                                         name=f"ash_{which}{h}_{lc}", tag="a")
                        nc.vector.stream_shuffle(out=ash, in_=a_t[h],
                                                 mask=SWAP_MASK)
                        dst = (q_sb if which == "q" else k_sb)[h]
                        sl = dst[:, ls_lo:ls_lo + 512]
                        nc.vector.tensor_sub(out=sl, in0=sl, in1=ash)

                def v_pass():
                    # v pass (x tiles stationary -> natural [l, d] layout);
                    # e-outer so each wv chunk is consumed once, in its DMA
                    # arrival order
                    accv = [acc_tile(f"pv_{lc}_{ls}") for ls in range(4)]
                    for e in range(NE):
                        for ls in range(4):
                            nc.tensor.matmul(
                                accv[ls],
                                lhsT=xsl(e, ls * 128, (ls + 1) * 128),
                                rhs=wv_sb[:, e, :],
                                start=(e == 0), stop=(e == NE - 1))
                    for ls in range(4):
                        if ls % 2 == 0:
                            nc.scalar.copy(out=v_sb[:, lc * 4 + ls, :],
                                           in_=accv[ls])
                        else:
                            nc.vector.tensor_copy(out=v_sb[:, lc * 4 + ls, :],
                                                  in_=accv[ls])

                if lc == 0:
                    acc8 = qk_pass((0, 1))
                    accq, acck = acc8[:HPG], acc8[HPG:]
                    aq = rope_a(accq, "q")
                    ak = rope_a(acck, "k")
                    rope_b(aq, "q")
                    v_pass()
                    rope_b(ak, "k")
                elif lc == NLC - 1:
                    # last chunk: finalize k before the v pass so attention's
                    # first score matmuls aren't gated on the v matmuls
                    accq = qk_pass((0,))
                    aq = rope_a(accq, "q")
                    acck = qk_pass((1,))
                    rope_b(aq, "q")
                    ak = rope_a(acck, "k")
                    rope_b(ak, "k")
                    v_pass()
                else:
                    accq = qk_pass((0,))
                    aq = rope_a(accq, "q")
                    acck = qk_pass((1,))
                    rope_b(aq, "q")
                    ak = rope_a(acck, "k")
                    v_pass()
                    rope_b(ak, "k")

        # ======== phase 2+3: causal attention with interleaved projection ==
        # jobs are ic-major: once all 4 heads finished l-chunk ic, that
        # chunk's output projection is emitted immediately — it fills
        # attention pipeline bubbles and spreads the output DMA.
        nc.gpsimd.dma_start(
            out=masks_sb, in_=d["masks"].rearrange("p (r f) -> p r f", r=4))
        nc.gpsimd.dma_start(out=ones_sb, in_=d["ones"])
        with tc.tile_pool(name="pexp", bufs=10) as pexp, \
             tc.tile_pool(name="zacc", bufs=3) as zacc, \
             tc.tile_pool(name="zpool", bufs=3) as zpool, \
             tc.tile_pool(name="outst", bufs=3) as outst, \
             tc.tile_pool(name="psum2", bufs=1, space="PSUM") as ps2:
            jobs = [(h, ic) for ic in range(NLC) for h in range(HPG)]
            # per-job block order: diagonal blocks first (their mask-muls
            # land while the za chain is short), then the full blocks -- the
            # job tail is then pure adds and the DVE catches up before the
            # denominator matmuls.  Blocks are processed in PAIRS sharing one
            # [128, 1024] psum tile, halving the scalar engine's
            # per-activation overhead (exp throughput paces the big jobs).
            jseq = {ji: list(range(4 * ic, 4 * ic + 4)) + list(range(4 * ic))
                    for ji, (_h, ic) in enumerate(jobs)}
            steps = [(ji, pi)
                     for ji in range(len(jobs))
                     for pi in range(len(jseq[ji]) // 2)]
            LA = 1
            pss_map = {}
            psy_map = {}
            za_map = {}
            psz_map = {}
            # output-projection work queue: quarters (lcx, lt, fq) are
            # interleaved one-per-step into the FOLLOWING chunk's attention
            # jobs -- the projection is scalar-light and PE-heavy, which pads
            # each window so the scalar engine's exp stream keeps up
            pending_proj = []
            ot_map = {}
            proj_stride = [1]
            proj_tick = [0]

            def emit_proj_quarter(tag="po"):
                lcx, lt, fq = pending_proj.pop(0)
                l0 = lcx * 512 + lt * 128
                if fq == 0:
                    ot_map[(lcx, lt)] = outst.tile(
                        [128, E], mybir.dt.float16,
                        name=f"ot_{lcx}_{lt}", tag="ot")
                ot = ot_map[(lcx, lt)]
                po = ps2.tile([128, 512], FP32, name=f"po_{lcx}_{lt}_{fq}",
                              tag=tag, bufs=1)
                for hh in range(HPG):
                    nc.tensor.matmul(
                        po,
                        lhsT=y_sb[hh][:, l0:l0 + 128],
                        rhs=wout_sb[:, hh, fq * 512:(fq + 1) * 512],
                        start=(hh == 0), stop=(hh == HPG - 1))
                if fq < 3:
                    nc.vector.tensor_copy(
                        out=ot[:, fq * 512:(fq + 1) * 512], in_=po)
                else:
                    nc.scalar.copy(
                        out=ot[:, fq * 512:(fq + 1) * 512], in_=po)
                    eng = (nc.sync, nc.gpsimd, nc.scalar, nc.sync)[lt % 4]
                    eng.dma_start(out=d["out"][l0:l0 + 128, :],
                                  in_=ot_map.pop((lcx, lt)))

            def blk(ji, pos):
                _h, ic = jobs[ji]
                jb = jseq[ji][pos]
                # diagonal blocks (r >= 1) have no valid columns below
                # f = 128*r: compute only the valid column range
                r = jb - 4 * ic
                return jb, r, (r * 128 if r > 0 else 0)

            def emit_s(ji, pi):
                h, ic = jobs[ji]
                t = ps2.tile([128, 1024], FP32, name=f"pss_{ji}_{pi}",
                             tag="pss", bufs=2)
                for k in (0, 1):
                    jb, r, lo = blk(ji, 2 * pi + k)
                    nc.tensor.matmul(
                        t[:, 512 * k + lo:512 * (k + 1)],
                        lhsT=k_sb[h][:, jb * 128:(jb + 1) * 128],
                        rhs=q_sb[h][:, ic * 512 + lo:(ic + 1) * 512],
                        start=True, stop=True)
                pss_map[(ji, pi)] = t

            ptr = 0
            for idx, (ji, pi) in enumerate(steps):
                while ptr < len(steps) and ptr <= idx + LA:
                    emit_s(*steps[ptr])
                    ptr += 1
                h, ic = jobs[ji]
                npairs = (4 * ic + 4) // 2
                if ji == 1 and pi == 0:
                    # wout is first needed by the ic=0 projection (~30us into
                    # the attention phase): deferring its load here keeps it
                    # clear of the startup bandwidth crunch
                    nc.gpsimd.dma_start(
                        out=wout_sb,
                        in_=d["wout"].rearrange("(h p) f -> p h f", p=128))
                if pi == 0:
                    psy_map[ji] = ps2.tile([128, 512], FP32, name=f"psy_{ji}",
                                           tag="psy", bufs=2)
                psy = psy_map[ji]
                pss = pss_map.pop((ji, pi))
                jbA, rA, loA = blk(ji, 2 * pi)
                jbB, rB, loB = blk(ji, 2 * pi + 1)
                pt = pexp.tile([128, 1024], FP16, name=f"pt_{ji}_{pi}",
                               tag="pexp", bufs=5)
                # one exp covers both halves (the dead zone [512:512+loB) of
                # a diagonal pair holds exp(stale psum) -- never read)
                nc.scalar.activation(out=pt[:, loA:], in_=pss[:, loA:],
                                     func=EXP, bias=ebias_sb)
                for k, r, lo in ((0, rA, loA), (1, rB, loB)):
                    if r >= 0:
                        # diagonal block: only the first 128 columns of the
                        # valid range hold the per-element triangle; the rest
                        # are all-1.  On gpsimd -- the DVE runs the za chain.
                        nc.gpsimd.tensor_mul(
                            out=pt[:, 512 * k + lo:512 * k + lo + 128],
                            in0=pt[:, 512 * k + lo:512 * k + lo + 128],
                            in1=masks_sb[:, r, lo:lo + 128])
                # running probability sum for the softmax denominator (DVE);
                # the final pair is NOT accumulated -- it goes straight into
                # the trailing denominator matmuls, so the PE never waits on
                # the full exp->add chain at the job end
                if pi == 0:
                    za = zacc.tile([128, 512], FP16, name=f"za_{ji}", tag="za")
                    za_map[ji] = za
                    nc.vector.tensor_copy(out=za, in_=pt[:, 0:512])
                    nc.vector.tensor_add(out=za[:, loB:], in0=za[:, loB:],
                                         in1=pt[:, 512 + loB:])
                elif pi < npairs - 1:
                    za = za_map[ji]
                    nc.vector.tensor_add(out=za[:, loA:], in0=za[:, loA:],
                                         in1=pt[:, loA:512])
                    nc.vector.tensor_add(out=za[:, loB:], in0=za[:, loB:],
                                         in1=pt[:, 512 + loB:])
                if pi == npairs - 2:
                    # all but the final pair accumulated: emit the first
                    # denominator matmul now (also broadcasts Z across
                    # partitions)
                    psz = ps2.tile([128, 512], FP32, name=f"psz_{ji}",
                                   tag="psz", bufs=1)
                    psz_map[ji] = psz
                    nc.tensor.matmul(psz, lhsT=ones_sb, rhs=za_map.pop(ji),
                                     start=True, stop=False)
                nc.tensor.matmul(psy[:, loA:],
                                 lhsT=v_sb[:, jbA, h * 128:(h + 1) * 128],
                                 rhs=pt[:, loA:512],
                                 start=(pi == 0), stop=False)
                nc.tensor.matmul(psy[:, loB:],
                                 lhsT=v_sb[:, jbB, h * 128:(h + 1) * 128],
                                 rhs=pt[:, 512 + loB:],
                                 start=False, stop=(pi == npairs - 1))
                if pi == npairs - 1:
                    # fold the final pair's probabilities into Z directly
                    psz = psz_map.pop(ji)
                    nc.tensor.matmul(psz[:, loA:], lhsT=ones_sb,
                                     rhs=pt[:, loA:512],
                                     start=False, stop=False)
                    nc.tensor.matmul(psz[:, loB:], lhsT=ones_sb,
                                     rhs=pt[:, 512 + loB:],
                                     start=False, stop=True)
                    zv = zpool.tile([128, 512], FP32, name=f"zinv_{ji}",
                                    tag="zinv")
                    nc.vector.reciprocal_approx_fast(out=zv, in_=psz)
                    nc.vector.tensor_mul(
                        out=y_sb[h][:, ic * 512:(ic + 1) * 512],
                        in0=psy_map.pop(ji), in1=zv)
                    if h == HPG - 1:
                        # all heads done for this l-chunk: queue its output
                        # projection ([l, f] orientation, full-E rows so the
                        # store DMA moves 4 KB per partition line); quarters
                        # are drained one per following step
                        pending_proj.extend(
                            (ic, lt, fq) for lt in range(4) for fq in range(4))
                        # spread the 16 quarters evenly over the next chunk's
                        # pair-steps so every following job gets PE padding
                        if ic + 1 < NLC:
                            nsteps = HPG * (4 * (ic + 1) + 4) // 2
                            proj_stride[0] = max(1, nsteps // 16)
                        else:
                            proj_stride[0] = 1
                        proj_tick[0] = 0
                if pending_proj:
                    proj_tick[0] += 1
                    if proj_tick[0] % proj_stride[0] == 0:
                        emit_proj_quarter()

            # final chunk's projection has no following steps: drain it,
            # alternating psum tags (the psz tag is free by now) so
            # back-to-back quarters don't serialize on one bank
            qn = 0
            while pending_proj:
                emit_proj_quarter("po" if qn % 2 == 0 else "psz")
                qn += 1


# ------------------------------------------------------------------ host side

# head-dim permutation: quadrant q holds rope pairs 16q..16q+15 as
# (even dims | odd dims), so the rope partner swap stays within a
# 32-partition quadrant (STREAM_SHUFFLE's reach)
_PERM_IDX = np.concatenate(
    [np.concatenate([np.arange(16) * 2 + 32 * q,
                     np.arange(16) * 2 + 1 + 32 * q])
     for q in range(4)])


def prep_in_maps(x, rope, w_attn, w_proj):
    x = np.asarray(x, np.float32)
    rope = np.asarray(rope, np.float32)
    w_attn = np.asarray(w_attn, np.float32)
    w_proj = np.asarray(w_proj, np.float32)

    sin = rope[:, :, 0].T                    # [64, L]
    cos = rope[:, :, 1].T
    cs = np.zeros((128, L), np.float32)
    ss = np.zeros((128, L), np.float32)
    for q in range(4):
        pr = slice(16 * q, 16 * (q + 1))     # pair indices of quadrant q
        cs[32 * q:32 * q + 16] = cos[pr]
        cs[32 * q + 16:32 * q + 32] = cos[pr]
        ss[32 * q:32 * q + 16] = -sin[pr]
        ss[32 * q + 16:32 * q + 32] = sin[pr]
    cs = (cs * SCALE).astype(BF16)
    ss = (ss * SCALE).astype(BF16)

    p = np.arange(128)[:, None]
    f = np.arange(512)[None, :]
    masks = np.zeros((128, 4, 512), np.float32)
    for r in range(4):
        masks[:, r, :] = (r * 128 + p <= f).astype(np.float32)
    masks = masks.reshape(128, 4 * 512).astype(np.float16)

    ones = np.ones((128, 128), np.float16)

    xt_b = [np.ascontiguousarray(x[b].T).astype(BF16) for b in range(B)]

    wqk_g, wv_g, wout_g = {}, {}, {}
    for g in range(G):
        heads = [g * HPG + hl for hl in range(HPG)]
        wq = [np.ascontiguousarray(
                 w_attn[h * 128:(h + 1) * 128, :][_PERM_IDX, :].T) for h in heads]
        wk = [np.ascontiguousarray(
                 w_attn[E + h * 128:E + (h + 1) * 128, :][_PERM_IDX, :].T)
              for h in heads]
        wqk_g[g] = np.concatenate(wq + wk, axis=1).astype(BF16)        # [E, 1024]
        wv_g[g] = np.concatenate(
            [w_attn[2 * E + h * 128:2 * E + (h + 1) * 128, :].T for h in heads],
            axis=1).astype(BF16)                                        # [E, 512]
        wout_g[g] = np.ascontiguousarray(
            w_proj[:, g * 512:(g + 1) * 512].T).astype(BF16)            # [512, E]

    in_maps = []
    for c in range(NCORES):
        b, g = divmod(c, G)
        in_maps.append({
            "xt": xt_b[b],
            "wqk": wqk_g[g],
            "wv": wv_g[g],
            "wout": wout_g[g],
            "cs": cs,
            "ss": ss,
            "masks": masks,
            "ones": ones,
        })
    return in_maps


def assemble_output(results):
    out = np.zeros((B, L, E), np.float32)
    for c in range(NCORES):
        b, g = divmod(c, G)
        out[b] += results[c]["out"]
    return out


_NC = None


def get_nc():
    global _NC
    if _NC is None:
        _NC = build_nc()
    return _NC


def run(x, rope, w_attn, w_proj, trace=False, tmpdir=None):
    nc = get_nc()
    in_maps = prep_in_maps(x, rope, w_attn, w_proj)
    kwargs = {}
    if trace:
        import sys
        import types
        from concourse import bass_utils as _bu
        try:
            from trn_agent_boot.trn_boot import _ntff_profile_via_ctypes
            hook = _ntff_profile_via_ctypes("/opt/axon/libaxon_pjrt.so")
            mod = types.ModuleType("antenv.axon_hooks")
            mod.get_axon_ntff_profile_hook = lambda: hook
            sys.modules["antenv.axon_hooks"] = mod
            _bu.upload_artifacts = lambda dd: dd
        except Exception as e:  # pragma: no cover
            print("trace hook unavailable:", e)
        kwargs = dict(trace=True, tmpdir=tmpdir)
    res = run_bass_kernel_spmd(nc, in_maps, core_ids=list(range(NCORES)), **kwargs)
    return assemble_output(res.results), res


def kernel(x, rope, w_attn, w_proj):
    out, _ = run(x, rope, w_attn, w_proj, trace=False)
    return out


# revision 44
# speedup vs baseline: 1.1982x; 1.0080x over previous
"""Causal self-attention (B=2, L=2048, E=2048, H=16, HD=128) on 8 trn2 cores.

Sharding: core c = (b, g) with b = c // 4 (batch), g = c % 4 (head group of 4).
Each core computes QKV projection for its 4 heads on its batch, causal
attention with RoPE, and a partial output projection (its heads' slice of
w_proj rows). Host sums the 4 partial projections per batch.

All matmuls run in bf16/fp16 with fp32 PSUM accumulation.

Key device-side structure (per core):
  - phase 1, per 512-wide l-chunk: q/k/v projections as K-accumulated
    matmuls; rope fused right behind each q/k chunk entirely on the DVE:
        rot = (q * cs) - qshuffle(q * ss)
    where qshuffle is a 32-partition-quadrant half swap (STREAM_SHUFFLE);
    the head-dim rows are permuted host-side so each rope pair partner
    lives in the same quadrant.  cs/ss are host-prebuilt [128, L] tables
    (softmax scale folded in).
  - phase 2: scores computed transposed (sT[j,i] = k_j . q_i) so P@V needs
    no transpose; softmax without max-subtraction; exp emitted in fp16
    scaled by 2^-10 (activation bias = -ln 1024) so probabilities can be
    block-accumulated on the DVE; the softmax denominator is ONE ones-matmul
    per (head, l-chunk) on the accumulated tile (also broadcasts Z across
    partitions); causal masking by skipping upper-triangle blocks + 4
    static diagonal masks; software-pipelined with a 3-deep score-matmul
    lookahead.
  - phase 3: partial out-projection in [l, f] orientation (lhsT = y tile,
    rhs = w_proj rows) emitted per l-chunk as soon as its 4 heads finish;
    stores are full-E rows ([128, 2048] fp16, 4 KB per partition line).

Device layouts (per core):
  xt    [E=2048, L=2048] bf16   x[b].T  (e on rows); loaded as [128, 1024]
                                tiles (2 KB DMA lines)
  wqk   [E, 1024]        bf16   8 col-blocks: q-heads 0..3, k-heads 0..3,
                                head rows perm'd quadrant-pairwise, transposed
  wv    [E, 512]         bf16   v weights, natural order, transposed
  wout  [512, E]         bf16   w_proj[:, g*512:(g+1)*512].T
  cs,ss [128, L]         bf16   rope cos / (-sin|+sin per quadrant) tables
                                * 128**-0.25
  masks [128, 4*512]     fp16   causal diagonal-block masks
  ones  [128, 128]       fp16   all-ones (softmax denominator broadcast-sum)
Output:
  out   [L, E] fp16  (partial projection; host adds in fp32)
"""

from contextlib import ExitStack

import numpy as np
import ml_dtypes

import concourse.bass as bass
import concourse.mybir as mybir
import concourse.tile as tile
from concourse import bacc
from concourse.bass_utils import run_bass_kernel_spmd

BF16 = ml_dtypes.bfloat16
B, L, E, H, HD = 2, 2048, 2048, 16, 128
G = 4            # head groups (cores per batch)
HPG = H // G     # heads per group = 4
NCORES = 8
NE = E // 128    # 16 e-chunks
NLC = L // 512   # 4 l-chunks of 512
NLT = L // 128   # 16 l-tiles of 128
SCALE = float(128.0 ** -0.25)   # per-operand score scale (q and k each)
EXPBIAS = -6.931471805599453    # -ln(1024): p scaled 2^-10, cancels in y=psy/Z

FP32 = mybir.dt.float32
FP16 = mybir.dt.float16
BF = mybir.dt.bfloat16

# rope partner swap within each 32-partition quadrant (i <-> i+16)
SWAP_MASK = list(range(16, 32)) + list(range(0, 16))


def build_nc():
    nc = bacc.Bacc(
        "TRN2",
        target_bir_lowering=False,
        debug=False,
        enable_asserts=False,
        num_devices=NCORES,
    )
    d = {}
    d["xt"] = nc.dram_tensor("xt", [E, L], BF, kind="ExternalInput").ap()
    d["wqk"] = nc.dram_tensor("wqk", [E, 2 * HPG * 128], BF, kind="ExternalInput").ap()
    d["wv"] = nc.dram_tensor("wv", [E, HPG * 128], BF, kind="ExternalInput").ap()
    d["wout"] = nc.dram_tensor("wout", [HPG * 128, E], BF, kind="ExternalInput").ap()
    d["cs"] = nc.dram_tensor("cs", [128, L], BF, kind="ExternalInput").ap()
    d["ss"] = nc.dram_tensor("ss", [128, L], BF, kind="ExternalInput").ap()
    d["masks"] = nc.dram_tensor("masks", [128, 4 * 512], FP16,
                                kind="ExternalInput").ap()
    d["ones"] = nc.dram_tensor("ones", [128, 128], FP16, kind="ExternalInput").ap()
    d["out"] = nc.dram_tensor("out", [L, E], mybir.dt.float16,
                              kind="ExternalOutput").ap()

    with tile.TileContext(nc) as tc:
        build_kernel(tc, d)
    nc.compile()
    return nc


def build_kernel(tc, d):
    nc = tc.nc
    EXP = mybir.ActivationFunctionType.Exp

    with ExitStack() as ctx:
        const = ctx.enter_context(tc.tile_pool(name="const", bufs=1))
        qkres = ctx.enter_context(tc.tile_pool(name="qkres", bufs=1))
        vres = ctx.enter_context(tc.tile_pool(name="vres", bufs=1))
        yres = ctx.enter_context(tc.tile_pool(name="yres", bufs=1))

        # ---- resident constants / weights ----
        cs_sb = const.tile([128, L], BF, name="cs_sb", tag="cs_sb")
        ss_sb = const.tile([128, L], BF, name="ss_sb", tag="ss_sb")
        wv_sb = const.tile([128, NE, HPG * 128], BF, name="wv_sb", tag="wv_sb")
        wv_r = d["wv"].rearrange("(ec p) f -> p ec f", p=128)
        # masks/ones/wout are needed only from the attention phase on; their
        # DMAs are emitted after phase 1 so they don't contend with the
        # startup-critical weight/x loads. Tiles allocated here.
        masks_sb = const.tile([128, 4, 512], FP16, name="masks_sb", tag="masks_sb")
        ones_sb = const.tile([128, 128], FP16, name="ones_sb", tag="ones_sb")
        wout_sb = const.tile([128, HPG, E], BF, name="wout_sb", tag="wout_sb")
        ebias_sb = const.tile([128, 1], FP32, name="ebias_sb", tag="ebias_sb")
        nc.vector.memset(ebias_sb, EXPBIAS)

        # ---- residents ----
        q_sb = [qkres.tile([128, L], BF, name=f"q_sb{h}", tag=f"q_sb{h}")
                for h in range(HPG)]
        k_sb = [qkres.tile([128, L], BF, name=f"k_sb{h}", tag=f"k_sb{h}")
                for h in range(HPG)]
        v_sb = vres.tile([128, NLT, HPG * 128], FP16, name="v_sb", tag="v_sb")
        y_sb = [yres.tile([128, L], BF, name=f"y_sb{h}", tag=f"y_sb{h}")
                for h in range(HPG)]

        # ================= phase 1: QKV projection + fused rope ============
        # phase-1-only pools (wqk weights, x tiles, rope temporaries): closed
        # after phase 1 so their SBUF is reusable by the attention pools.
        with tc.tile_pool(name="wqks", bufs=1) as wqks, \
             tc.tile_pool(name="xs", bufs=24) as xs, \
             tc.tile_pool(name="atile", bufs=12) as atile, \
             tc.tile_pool(name="psum1", bufs=1, space="PSUM") as ps1:

            wqk_sb = wqks.tile([128, NE, 2 * HPG * 128], BF, name="wqk_sb",
                               tag="wqk_sb")
            wqk_r = d["wqk"].rearrange("(ec p) f -> p ec f", p=128)
            # All wqk on the gpsimd queue: first chunks as singles (earliest
            # arrival), the rest as pairs (amortizes the ~0.5us per-dma ring
            # overhead so delivery keeps up with the PE's ~1.7us/chunk
            # consumption).  x tiles split across sync+scalar (see below) --
            # aggregate HBM bandwidth is the startup constraint.
            # chunk 0 rides at the head of the sync queue, which starts
            # earliest -- this de-jitters the first real matmul
            nc.sync.dma_start(out=wqk_sb[:, 0, :], in_=wqk_r[:, 0, :])
            for e in range(1, 4):
                nc.gpsimd.dma_start(out=wqk_sb[:, e, :], in_=wqk_r[:, e, :])
            for e in range(4, NE, 2):
                nc.gpsimd.dma_start(out=wqk_sb[:, e:e + 2, :],
                                    in_=wqk_r[:, e:e + 2, :])

            def acc_tile(nm):
                return ps1.tile([128, 512], FP32, name=nm, tag="pacc", bufs=8)

            # PE warmup: dummy matmuls on never-written SBUF fill the idle
            # window while the first weights stream in, so the tensor engine's
            # clock ramp completes before real work starts (results unused)
            dummy_sb = wqks.tile([128, 512], BF, name="dummy_sb", tag="dummy")
            nc.vector.memset(dummy_sb, 0)
            for i in range(8):
                wt = acc_tile(f"warm{i}")
                nc.tensor.matmul(wt, lhsT=dummy_sb[:, :128], rhs=dummy_sb,
                                 start=True, stop=True)

            xt_pair = []  # [128, 1024] tiles covering l-chunks (2p, 2p+1)

            for lc in range(NLC):
                ls_lo = lc * 512
                cs_lc = cs_sb[:, ls_lo:ls_lo + 512]
                ss_lc = ss_sb[:, ls_lo:ls_lo + 512]

                if lc % 2 == 0:
                    # x tiles loaded 1024 wide (2 KB per partition line).
                    # Startup (pair 0): even e on sync, odd e on scalar so the
                    # two streams keep the PE's e-cadence.  Pair 1 is not
                    # needed until ~100us, so it all goes on scalar BEHIND wv,
                    # clearing the startup bandwidth crunch.
                    xt_pair = []
                    for e in range(NE):
                        t = xs.tile([128, 1024], BF, name=f"xt_{lc}_{e}",
                                    tag="xt")
                        if lc == 0:
                            eng = nc.sync if e % 2 == 0 else nc.scalar
                        else:
                            eng = nc.scalar
                        eng.dma_start(
                            out=t,
                            in_=d["xt"][e * 128:(e + 1) * 128,
                                        ls_lo:ls_lo + 1024])
                        xt_pair.append(t)
                    if lc == 0:
                        # rope tables / v weights ride behind the first x
                        # half-streams on their respective queues
                        nc.sync.dma_start(out=cs_sb, in_=d["cs"])
                        nc.sync.dma_start(out=ss_sb, in_=d["ss"])
                        for e in range(0, NE, 2):
                            nc.scalar.dma_start(out=wv_sb[:, e:e + 2, :],
                                                in_=wv_r[:, e:e + 2, :])
                sub = (lc % 2) * 512

                def xsl(e, a, b):
                    # slice of this l-chunk's half of the [128, 1024] x tile
                    return xt_pair[e][:, sub + a:sub + b]

                def qk_pass(halves):
                    # projection matmuls for the given f-block halves
                    # (0 = q heads, 1 = k heads); passing both interleaves
                    # them per e-chunk, which halves the weight-chunk arrival
                    # rate the PE needs (used for the DMA-bound first l-chunk)
                    acc = [acc_tile(f"p{half}_{lc}_{h}")
                           for half in halves for h in range(HPG)]
                    for e in range(NE):
                        for i, half in enumerate(halves):
                            for h in range(HPG):
                                fb = half * HPG + h
                                nc.tensor.matmul(
                                    acc[i * HPG + h],
                                    lhsT=wqk_sb[:, e, fb * 128:(fb + 1) * 128],
                                    rhs=xsl(e, 0, 512),
                                    start=(e == 0), stop=(e == NE - 1))
                    return acc

                def rope_a(acc, which):
                    # a = q*ss (bf16), dst-slice = q*cs ; releases acc
                    a_t = []
                    for h in range(HPG):
                        a = atile.tile([128, 512], BF,
                                       name=f"a_{which}{h}_{lc}", tag="a")
                        nc.vector.tensor_mul(out=a, in0=acc[h], in1=ss_lc)
                        dst = (q_sb if which == "q" else k_sb)[h]
                        nc.vector.tensor_mul(
                            out=dst[:, ls_lo:ls_lo + 512], in0=acc[h], in1=cs_lc)
                        a_t.append(a)
                    return a_t

                def rope_b(a_t, which):
                    # dst -= quadrant_swap(a)   (all on DVE)
                    for h in range(HPG):
                        ash = atile.tile([128, 512], BF,
                                         name=f"ash_{which}{h}_{lc}", tag="a")
                        nc.vector.stream_shuffle(out=ash, in_=a_t[h],
                                                 mask=SWAP_MASK)
                        dst = (q_sb if which == "q" else k_sb)[h]
                        sl = dst[:, ls_lo:ls_lo + 512]
                        nc.vector.tensor_sub(out=sl, in0=sl, in1=ash)

                def v_pass():
                    # v pass (x tiles stationary -> natural [l, d] layout);
                    # e-outer so each wv chunk is consumed once, in its DMA
                    # arrival order
                    accv = [acc_tile(f"pv_{lc}_{ls}") for ls in range(4)]
                    for e in range(NE):
                        for ls in range(4):
                            nc.tensor.matmul(
                                accv[ls],
                                lhsT=xsl(e, ls * 128, (ls + 1) * 128),
                                rhs=wv_sb[:, e, :],
                                start=(e == 0), stop=(e == NE - 1))
                    for ls in range(4):
                        if ls % 2 == 0:
                            nc.scalar.copy(out=v_sb[:, lc * 4 + ls, :],
                                           in_=accv[ls])
                        else:
                            nc.vector.tensor_copy(out=v_sb[:, lc * 4 + ls, :],
                                                  in_=accv[ls])

                if lc == 0:
                    acc8 = qk_pass((0, 1))
                    accq, acck = acc8[:HPG], acc8[HPG:]
                    aq = rope_a(accq, "q")
                    ak = rope_a(acck, "k")
                    rope_b(aq, "q")
                    v_pass()
                    rope_b(ak, "k")
                elif lc == NLC - 1:
                    # last chunk: finalize k before the v pass so attention's
                    # first score matmuls aren't gated on the v matmuls
                    accq = qk_pass((0,))
                    aq = rope_a(accq, "q")
                    acck = qk_pass((1,))
                    rope_b(aq, "q")
                    ak = rope_a(acck, "k")
                    rope_b(ak, "k")
                    v_pass()
                else:
                    accq = qk_pass((0,))
                    aq = rope_a(accq, "q")
                    acck = qk_pass((1,))
                    rope_b(aq, "q")
                    ak = rope_a(acck, "k")
                    v_pass()
                    rope_b(ak, "k")

        # ======== phase 2+3: causal attention with interleaved projection ==
        # jobs are ic-major: once all 4 heads finished l-chunk ic, that
        # chunk's output projection is emitted immediately — it fills
        # attention pipeline bubbles and spreads the output DMA.
        nc.gpsimd.dma_start(
            out=masks_sb, in_=d["masks"].rearrange("p (r f) -> p r f", r=4))
        nc.gpsimd.dma_start(out=ones_sb, in_=d["ones"])
        with tc.tile_pool(name="pexp", bufs=10) as pexp, \
             tc.tile_pool(name="zacc", bufs=3) as zacc, \
             tc.tile_pool(name="zpool", bufs=3) as zpool, \
             tc.tile_pool(name="outst", bufs=3) as outst, \
             tc.tile_pool(name="psum2", bufs=1, space="PSUM") as ps2:
            jobs = [(h, ic) for ic in range(NLC) for h in range(HPG)]
            # per-job block order: diagonal blocks first (their mask-muls
            # land while the za chain is short), then the full blocks -- the
            # job tail is then pure adds and the DVE catches up before the
            # denominator matmuls.  Blocks are processed in PAIRS sharing one
            # [128, 1024] psum tile, halving the scalar engine's
            # per-activation overhead (exp throughput paces the big jobs).
            jseq = {ji: list(range(4 * ic, 4 * ic + 4)) + list(range(4 * ic))
                    for ji, (_h, ic) in enumerate(jobs)}
            steps = [(ji, pi)
                     for ji in range(len(jobs))
                     for pi in range(len(jseq[ji]) // 2)]
            LA = 1
            pss_map = {}
            psy_map = {}
            za_map = {}
            psz_map = {}
            # output-projection work queue: quarters (lcx, lt, fq) are
            # interleaved one-per-step into the FOLLOWING chunk's attention
            # jobs -- the projection is scalar-light and PE-heavy, which pads
            # each window so the scalar engine's exp stream keeps up
            pending_proj = []
            ot_map = {}
            proj_stride = [1]
            proj_tick = [0]

            def emit_proj_quarter(tag="po"):
                lcx, lt, fq = pending_proj.pop(0)
                l0 = lcx * 512 + lt * 128
                if fq == 0:
                    ot_map[(lcx, lt)] = outst.tile(
                        [128, E], mybir.dt.float16,
                        name=f"ot_{lcx}_{lt}", tag="ot")
                ot = ot_map[(lcx, lt)]
                po = ps2.tile([128, 512], FP32, name=f"po_{lcx}_{lt}_{fq}",
                              tag=tag, bufs=1)
                for hh in range(HPG):
                    nc.tensor.matmul(
                        po,
                        lhsT=y_sb[hh][:, l0:l0 + 128],
                        rhs=wout_sb[:, hh, fq * 512:(fq + 1) * 512],
                        start=(hh == 0), stop=(hh == HPG - 1))
                if fq < 3:
                    nc.vector.tensor_copy(
                        out=ot[:, fq * 512:(fq + 1) * 512], in_=po)
                else:
                    nc.scalar.copy(
                        out=ot[:, fq * 512:(fq + 1) * 512], in_=po)
                    eng = (nc.sync, nc.gpsimd, nc.scalar, nc.sync)[lt % 4]
                    eng.dma_start(out=d["out"][l0:l0 + 128, :],
                                  in_=ot_map.pop((lcx, lt)))

            def blk(ji, pos):
                _h, ic = jobs[ji]
                jb = jseq[ji][pos]
                # diagonal blocks (r >= 1) have no valid columns below
                # f = 128*r: compute only the valid column range
                r = jb - 4 * ic
                return jb, r, (r * 128 if r > 0 else 0)

            def emit_s(ji, pi):
                h, ic = jobs[ji]
                t = ps2.tile([128, 1024], FP32, name=f"pss_{ji}_{pi}",
                             tag="pss", bufs=2)
                for k in (0, 1):
                    jb, r, lo = blk(ji, 2 * pi + k)
                    nc.tensor.matmul(
                        t[:, 512 * k + lo:512 * (k + 1)],
                        lhsT=k_sb[h][:, jb * 128:(jb + 1) * 128],
                        rhs=q_sb[h][:, ic * 512 + lo:(ic + 1) * 512],
                        start=True, stop=True)
                pss_map[(ji, pi)] = t

            ptr = 0
            for idx, (ji, pi) in enumerate(steps):
                while ptr < len(steps) and ptr <= idx + LA:
                    emit_s(*steps[ptr])
                    ptr += 1
                h, ic = jobs[ji]
                npairs = (4 * ic + 4) // 2
                if ji == 1 and pi == 0:
                    # wout is first needed by the ic=0 projection (~30us into
                    # the attention phase): deferring its load here keeps it
                    # clear of the startup bandwidth crunch
                    nc.gpsimd.dma_start(
                        out=wout_sb,
                        in_=d["wout"].rearrange("(h p) f -> p h f", p=128))
                if pi == 0:
                    psy_map[ji] = ps2.tile([128, 512], FP32, name=f"psy_{ji}",
                                           tag="psy", bufs=2)
                psy = psy_map[ji]
                pss = pss_map.pop((ji, pi))
                jbA, rA, loA = blk(ji, 2 * pi)
                jbB, rB, loB = blk(ji, 2 * pi + 1)
                pt = pexp.tile([128, 1024], FP16, name=f"pt_{ji}_{pi}",
                               tag="pexp", bufs=5)
                # one exp covers both halves (the dead zone [512:512+loB) of
                # a diagonal pair holds exp(stale psum) -- never read)
                nc.scalar.activation(out=pt[:, loA:], in_=pss[:, loA:],
                                     func=EXP, bias=ebias_sb)
                for k, r, lo in ((0, rA, loA), (1, rB, loB)):
                    if r >= 0:
                        # diagonal block: only the first 128 columns of the
                        # valid range hold the per-element triangle; the rest
                        # are all-1.  On gpsimd -- the DVE runs the za chain.
                        nc.gpsimd.tensor_mul(
                            out=pt[:, 512 * k + lo:512 * k + lo + 128],
                            in0=pt[:, 512 * k + lo:512 * k + lo + 128],
                            in1=masks_sb[:, r, lo:lo + 128])
                # running probability sum for the softmax denominator (DVE);
                # the final pair is NOT accumulated -- it goes straight into
                # the trailing denominator matmuls, so the PE never waits on
                # the full exp->add chain at the job end
                if pi == 0:
                    za = zacc.tile([128, 512], FP16, name=f"za_{ji}", tag="za")
                    za_map[ji] = za
                    nc.vector.tensor_copy(out=za, in_=pt[:, 0:512])
                    nc.vector.tensor_add(out=za[:, loB:], in0=za[:, loB:],
                                         in1=pt[:, 512 + loB:])
                elif pi < npairs - 1:
                    za = za_map[ji]
                    nc.vector.tensor_add(out=za[:, loA:], in0=za[:, loA:],
                                         in1=pt[:, loA:512])
                    nc.vector.tensor_add(out=za[:, loB:], in0=za[:, loB:],
                                         in1=pt[:, 512 + loB:])
                if pi == npairs - 2:
                    # all but the final pair accumulated: emit the first
                    # denominator matmul now (also broadcasts Z across
                    # partitions)
                    psz = ps2.tile([128, 512], FP32, name=f"psz_{ji}",
                                   tag="psz", bufs=1)
                    psz_map[ji] = psz
                    nc.tensor.matmul(psz, lhsT=ones_sb, rhs=za_map.pop(ji),
                                     start=True, stop=False)
                nc.tensor.matmul(psy[:, loA:],
                                 lhsT=v_sb[:, jbA, h * 128:(h + 1) * 128],
                                 rhs=pt[:, loA:512],
                                 start=(pi == 0), stop=False)
                nc.tensor.matmul(psy[:, loB:],
                                 lhsT=v_sb[:, jbB, h * 128:(h + 1) * 128],
                                 rhs=pt[:, 512 + loB:],
                                 start=False, stop=(pi == npairs - 1))
                if pi == npairs - 1:
                    # fold the final pair's probabilities into Z directly
                    psz = psz_map.pop(ji)
                    nc.tensor.matmul(psz[:, loA:], lhsT=ones_sb,
                                     rhs=pt[:, loA:512],
                                     start=False, stop=False)
                    nc.tensor.matmul(psz[:, loB:], lhsT=ones_sb,
                                     rhs=pt[:, 512 + loB:],
                                     start=False, stop=True)
                    zv = zpool.tile([128, 512], FP32, name=f"zinv_{ji}",
                                    tag="zinv")
                    nc.vector.reciprocal_approx_fast(out=zv, in_=psz)
                    nc.vector.tensor_mul(
                        out=y_sb[h][:, ic * 512:(ic + 1) * 512],
                        in0=psy_map.pop(ji), in1=zv)
                    if h == HPG - 1:
                        # all heads done for this l-chunk: queue its output
                        # projection ([l, f] orientation, full-E rows so the
                        # store DMA moves 4 KB per partition line); quarters
                        # are drained one per following step
                        pending_proj.extend(
                            (ic, lt, fq) for lt in range(4) for fq in range(4))
                        # spread the 16 quarters evenly over the next chunk's
                        # pair-steps so every following job gets PE padding
                        if ic + 1 < NLC:
                            nsteps = HPG * (4 * (ic + 1) + 4) // 2
                            proj_stride[0] = max(1, nsteps // 16)
                        else:
                            proj_stride[0] = 1
                        # start draining a couple of steps late so the first
                        # quarters don't wait on this chunk's final ymul
                        proj_tick[0] = -2
                if pending_proj:
                    proj_tick[0] += 1
                    if proj_tick[0] > 0 and proj_tick[0] % proj_stride[0] == 0:
                        emit_proj_quarter()

            # final chunk's projection has no following steps: drain it,
            # alternating psum tags (the psz tag is free by now) so
            # back-to-back quarters don't serialize on one bank
            qn = 0
            while pending_proj:
                emit_proj_quarter("po" if qn % 2 == 0 else "psz")
                qn += 1


# ------------------------------------------------------------------ host side

# head-dim permutation: quadrant q holds rope pairs 16q..16q+15 as
# (even dims | odd dims), so the rope partner swap stays within a
# 32-partition quadrant (STREAM_SHUFFLE's reach)
_PERM_IDX = np.concatenate(
    [np.concatenate([np.arange(16) * 2 + 32 * q,
                     np.arange(16) * 2 + 1 + 32 * q])
     for q in range(4)])


def prep_in_maps(x, rope, w_attn, w_proj):
    x = np.asarray(x, np.float32)
    rope = np.asarray(rope, np.float32)
    w_attn = np.asarray(w_attn, np.float32)
    w_proj = np.asarray(w_proj, np.float32)

    sin = rope[:, :, 0].T                    # [64, L]
    cos = rope[:, :, 1].T
    cs = np.zeros((128, L), np.float32)
    ss = np.zeros((128, L), np.float32)
    for q in range(4):
        pr = slice(16 * q, 16 * (q + 1))     # pair indices of quadrant q
        cs[32 * q:32 * q + 16] = cos[pr]
        cs[32 * q + 16:32 * q + 32] = cos[pr]
        ss[32 * q:32 * q + 16] = -sin[pr]
        ss[32 * q + 16:32 * q + 32] = sin[pr]
    cs = (cs * SCALE).astype(BF16)
    ss = (ss * SCALE).astype(BF16)

    p = np.arange(128)[:, None]
    f = np.arange(512)[None, :]
    masks = np.zeros((128, 4, 512), np.float32)
    for r in range(4):
        masks[:, r, :] = (r * 128 + p <= f).astype(np.float32)
    masks = masks.reshape(128, 4 * 512).astype(np.float16)

    ones = np.ones((128, 128), np.float16)

    xt_b = [np.ascontiguousarray(x[b].T).astype(BF16) for b in range(B)]

    wqk_g, wv_g, wout_g = {}, {}, {}
    for g in range(G):
        heads = [g * HPG + hl for hl in range(HPG)]
        wq = [np.ascontiguousarray(
                 w_attn[h * 128:(h + 1) * 128, :][_PERM_IDX, :].T) for h in heads]
        wk = [np.ascontiguousarray(
                 w_attn[E + h * 128:E + (h + 1) * 128, :][_PERM_IDX, :].T)
              for h in heads]
        wqk_g[g] = np.concatenate(wq + wk, axis=1).astype(BF16)        # [E, 1024]
        wv_g[g] = np.concatenate(
            [w_attn[2 * E + h * 128:2 * E + (h + 1) * 128, :].T for h in heads],
            axis=1).astype(BF16)                                        # [E, 512]
        wout_g[g] = np.ascontiguousarray(
            w_proj[:, g * 512:(g + 1) * 512].T).astype(BF16)            # [512, E]

    in_maps = []
    for c in range(NCORES):
        b, g = divmod(c, G)
        in_maps.append({
            "xt": xt_b[b],
            "wqk": wqk_g[g],
            "wv": wv_g[g],
            "wout": wout_g[g],
            "cs": cs,
            "ss": ss,
            "masks": masks,
            "ones": ones,
        })
    return in_maps


def assemble_output(results):
    out = np.zeros((B, L, E), np.float32)
    for c in range(NCORES):
        b, g = divmod(c, G)
        out[b] += results[c]["out"]
    return out


_NC = None


def get_nc():
    global _NC
    if _NC is None:
        _NC = build_nc()
    return _NC


def run(x, rope, w_attn, w_proj, trace=False, tmpdir=None):
    nc = get_nc()
    in_maps = prep_in_maps(x, rope, w_attn, w_proj)
    kwargs = {}
    if trace:
        import sys
        import types
        from concourse import bass_utils as _bu
        try:
            from trn_agent_boot.trn_boot import _ntff_profile_via_ctypes
            hook = _ntff_profile_via_ctypes("/opt/axon/libaxon_pjrt.so")
            mod = types.ModuleType("antenv.axon_hooks")
            mod.get_axon_ntff_profile_hook = lambda: hook
            sys.modules["antenv.axon_hooks"] = mod
            _bu.upload_artifacts = lambda dd: dd
        except Exception as e:  # pragma: no cover
            print("trace hook unavailable:", e)
        kwargs = dict(trace=True, tmpdir=tmpdir)
    res = run_bass_kernel_spmd(nc, in_maps, core_ids=list(range(NCORES)), **kwargs)
    return assemble_output(res.results), res


def kernel(x, rope, w_attn, w_proj):
    out, _ = run(x, rope, w_attn, w_proj, trace=False)
    return out


# revision 47
# speedup vs baseline: 1.2030x; 1.0040x over previous
"""Causal self-attention (B=2, L=2048, E=2048, H=16, HD=128) on 8 trn2 cores.

Sharding: core c = (b, g) with b = c // 4 (batch), g = c % 4 (head group of 4).
Each core computes QKV projection for its 4 heads on its batch, causal
attention with RoPE, and a partial output projection (its heads' slice of
w_proj rows). Host sums the 4 partial projections per batch.

All matmuls run in bf16/fp16 with fp32 PSUM accumulation.

Key device-side structure (per core):
  - phase 1, per 512-wide l-chunk: q/k/v projections as K-accumulated
    matmuls; rope fused right behind each q/k chunk entirely on the DVE:
        rot = (q * cs) - qshuffle(q * ss)
    where qshuffle is a 32-partition-quadrant half swap (STREAM_SHUFFLE);
    the head-dim rows are permuted host-side so each rope pair partner
    lives in the same quadrant.  cs/ss are host-prebuilt [128, L] tables
    (softmax scale folded in).
  - phase 2: scores computed transposed (sT[j,i] = k_j . q_i) so P@V needs
    no transpose; softmax without max-subtraction; exp emitted in fp16
    scaled by 2^-10 (activation bias = -ln 1024) so probabilities can be
    block-accumulated on the DVE; the softmax denominator is ONE ones-matmul
    per (head, l-chunk) on the accumulated tile (also broadcasts Z across
    partitions); causal masking by skipping upper-triangle blocks + 4
    static diagonal masks; software-pipelined with a 3-deep score-matmul
    lookahead.
  - phase 3: partial out-projection in [l, f] orientation (lhsT = y tile,
    rhs = w_proj rows) emitted per l-chunk as soon as its 4 heads finish;
    stores are full-E rows ([128, 2048] fp16, 4 KB per partition line).

Device layouts (per core):
  xt    [E=2048, L=2048] bf16   x[b].T  (e on rows); loaded as [128, 1024]
                                tiles (2 KB DMA lines)
  wqk   [E, 1024]        bf16   8 col-blocks: q-heads 0..3, k-heads 0..3,
                                head rows perm'd quadrant-pairwise, transposed
  wv    [E, 512]         bf16   v weights, natural order, transposed
  wout  [512, E]         bf16   w_proj[:, g*512:(g+1)*512].T
  cs,ss [128, L]         bf16   rope cos / (-sin|+sin per quadrant) tables
                                * 128**-0.25
  masks [128, 4*512]     fp16   causal diagonal-block masks
  ones  [128, 128]       fp16   all-ones (softmax denominator broadcast-sum)
Output:
  out   [L, E] fp16  (partial projection; host adds in fp32)
"""

from contextlib import ExitStack

import numpy as np
import ml_dtypes

import concourse.bass as bass
import concourse.mybir as mybir
import concourse.tile as tile
from concourse import bacc
from concourse.bass_utils import run_bass_kernel_spmd

BF16 = ml_dtypes.bfloat16
B, L, E, H, HD = 2, 2048, 2048, 16, 128
G = 4            # head groups (cores per batch)
HPG = H // G     # heads per group = 4
NCORES = 8
NE = E // 128    # 16 e-chunks
NLC = L // 512   # 4 l-chunks of 512
NLT = L // 128   # 16 l-tiles of 128
SCALE = float(128.0 ** -0.25)   # per-operand score scale (q and k each)
EXPBIAS = -6.931471805599453    # -ln(1024): p scaled 2^-10, cancels in y=psy/Z

FP32 = mybir.dt.float32
FP16 = mybir.dt.float16
BF = mybir.dt.bfloat16

# rope partner swap within each 32-partition quadrant (i <-> i+16)
SWAP_MASK = list(range(16, 32)) + list(range(0, 16))


def build_nc():
    nc = bacc.Bacc(
        "TRN2",
        target_bir_lowering=False,
        debug=False,
        enable_asserts=False,
        num_devices=NCORES,
    )
    d = {}
    d["xt"] = nc.dram_tensor("xt", [E, L], BF, kind="ExternalInput").ap()
    d["wqk"] = nc.dram_tensor("wqk", [E, 2 * HPG * 128], BF, kind="ExternalInput").ap()
    d["wv"] = nc.dram_tensor("wv", [E, HPG * 128], BF, kind="ExternalInput").ap()
    d["wout"] = nc.dram_tensor("wout", [HPG * 128, E], BF, kind="ExternalInput").ap()
    d["cs"] = nc.dram_tensor("cs", [128, L], BF, kind="ExternalInput").ap()
    d["ss"] = nc.dram_tensor("ss", [128, L], BF, kind="ExternalInput").ap()
    d["masks"] = nc.dram_tensor("masks", [128, 4 * 512], FP16,
                                kind="ExternalInput").ap()
    d["ones"] = nc.dram_tensor("ones", [128, 128], FP16, kind="ExternalInput").ap()
    d["out"] = nc.dram_tensor("out", [L, E], mybir.dt.float16,
                              kind="ExternalOutput").ap()

    with tile.TileContext(nc) as tc:
        build_kernel(tc, d)
    nc.compile()
    return nc


def build_kernel(tc, d):
    nc = tc.nc
    EXP = mybir.ActivationFunctionType.Exp

    with ExitStack() as ctx:
        const = ctx.enter_context(tc.tile_pool(name="const", bufs=1))
        qkres = ctx.enter_context(tc.tile_pool(name="qkres", bufs=1))
        vres = ctx.enter_context(tc.tile_pool(name="vres", bufs=1))
        yres = ctx.enter_context(tc.tile_pool(name="yres", bufs=1))

        # ---- resident constants / weights ----
        cs_sb = const.tile([128, L], BF, name="cs_sb", tag="cs_sb")
        ss_sb = const.tile([128, L], BF, name="ss_sb", tag="ss_sb")
        wv_sb = const.tile([128, NE, HPG * 128], BF, name="wv_sb", tag="wv_sb")
        wv_r = d["wv"].rearrange("(ec p) f -> p ec f", p=128)
        # masks/ones/wout are needed only from the attention phase on; their
        # DMAs are emitted after phase 1 so they don't contend with the
        # startup-critical weight/x loads. Tiles allocated here.
        masks_sb = const.tile([128, 4, 512], FP16, name="masks_sb", tag="masks_sb")
        ones_sb = const.tile([128, 128], FP16, name="ones_sb", tag="ones_sb")
        wout_sb = const.tile([128, HPG, E], BF, name="wout_sb", tag="wout_sb")
        ebias_sb = const.tile([128, 1], FP32, name="ebias_sb", tag="ebias_sb")
        nc.vector.memset(ebias_sb, EXPBIAS)

        # ---- residents ----
        q_sb = [qkres.tile([128, L], BF, name=f"q_sb{h}", tag=f"q_sb{h}")
                for h in range(HPG)]
        k_sb = [qkres.tile([128, L], BF, name=f"k_sb{h}", tag=f"k_sb{h}")
                for h in range(HPG)]
        v_sb = vres.tile([128, NLT, HPG * 128], FP16, name="v_sb", tag="v_sb")
        y_sb = [yres.tile([128, L], BF, name=f"y_sb{h}", tag=f"y_sb{h}")
                for h in range(HPG)]

        # ================= phase 1: QKV projection + fused rope ============
        # phase-1-only pools (wqk weights, x tiles, rope temporaries): closed
        # after phase 1 so their SBUF is reusable by the attention pools.
        with tc.tile_pool(name="wqks", bufs=1) as wqks, \
             tc.tile_pool(name="xs", bufs=24) as xs, \
             tc.tile_pool(name="atile", bufs=12) as atile, \
             tc.tile_pool(name="psum1", bufs=1, space="PSUM") as ps1:

            wqk_sb = wqks.tile([128, NE, 2 * HPG * 128], BF, name="wqk_sb",
                               tag="wqk_sb")
            wqk_r = d["wqk"].rearrange("(ec p) f -> p ec f", p=128)
            # All wqk on the gpsimd queue: first chunks as singles (earliest
            # arrival), the rest as pairs (amortizes the ~0.5us per-dma ring
            # overhead so delivery keeps up with the PE's ~1.7us/chunk
            # consumption).  x tiles split across sync+scalar (see below) --
            # aggregate HBM bandwidth is the startup constraint.
            # chunk 0 rides at the head of the sync queue, which starts
            # earliest -- this de-jitters the first real matmul
            nc.sync.dma_start(out=wqk_sb[:, 0, :], in_=wqk_r[:, 0, :])
            for e in range(1, 4):
                nc.gpsimd.dma_start(out=wqk_sb[:, e, :], in_=wqk_r[:, e, :])
            for e in range(4, NE, 2):
                nc.gpsimd.dma_start(out=wqk_sb[:, e:e + 2, :],
                                    in_=wqk_r[:, e:e + 2, :])

            def acc_tile(nm):
                return ps1.tile([128, 512], FP32, name=nm, tag="pacc", bufs=8)

            # PE warmup: dummy matmuls on never-written SBUF fill the idle
            # window while the first weights stream in, so the tensor engine's
            # clock ramp completes before real work starts (results unused)
            dummy_sb = wqks.tile([128, 512], BF, name="dummy_sb", tag="dummy")
            nc.vector.memset(dummy_sb, 0)
            for i in range(8):
                wt = acc_tile(f"warm{i}")
                nc.tensor.matmul(wt, lhsT=dummy_sb[:, :128], rhs=dummy_sb,
                                 start=True, stop=True)

            xt_pair = []  # [128, 1024] tiles covering l-chunks (2p, 2p+1)

            for lc in range(NLC):
                ls_lo = lc * 512
                cs_lc = cs_sb[:, ls_lo:ls_lo + 512]
                ss_lc = ss_sb[:, ls_lo:ls_lo + 512]

                if lc % 2 == 0:
                    # x tiles loaded 1024 wide (2 KB per partition line).
                    # Startup (pair 0): even e on sync, odd e on scalar so the
                    # two streams keep the PE's e-cadence.  Pair 1 is not
                    # needed until ~100us, so it all goes on scalar BEHIND wv,
                    # clearing the startup bandwidth crunch.
                    xt_pair = []
                    for e in range(NE):
                        t = xs.tile([128, 1024], BF, name=f"xt_{lc}_{e}",
                                    tag="xt")
                        if lc == 0:
                            eng = nc.sync if e % 2 == 0 else nc.scalar
                        else:
                            eng = nc.scalar
                        eng.dma_start(
                            out=t,
                            in_=d["xt"][e * 128:(e + 1) * 128,
                                        ls_lo:ls_lo + 1024])
                        xt_pair.append(t)
                    if lc == 0:
                        # rope tables / v weights ride behind the first x
                        # half-streams on their respective queues
                        nc.sync.dma_start(out=cs_sb, in_=d["cs"])
                        nc.sync.dma_start(out=ss_sb, in_=d["ss"])
                        for e in range(0, NE, 2):
                            nc.scalar.dma_start(out=wv_sb[:, e:e + 2, :],
                                                in_=wv_r[:, e:e + 2, :])
                sub = (lc % 2) * 512

                def xsl(e, a, b):
                    # slice of this l-chunk's half of the [128, 1024] x tile
                    return xt_pair[e][:, sub + a:sub + b]

                def qk_pass(halves):
                    # projection matmuls for the given f-block halves
                    # (0 = q heads, 1 = k heads); passing both interleaves
                    # them per e-chunk, which halves the weight-chunk arrival
                    # rate the PE needs (used for the DMA-bound first l-chunk)
                    acc = [acc_tile(f"p{half}_{lc}_{h}")
                           for half in halves for h in range(HPG)]
                    for e in range(NE):
                        for i, half in enumerate(halves):
                            for h in range(HPG):
                                fb = half * HPG + h
                                nc.tensor.matmul(
                                    acc[i * HPG + h],
                                    lhsT=wqk_sb[:, e, fb * 128:(fb + 1) * 128],
                                    rhs=xsl(e, 0, 512),
                                    start=(e == 0), stop=(e == NE - 1))
                    return acc

                def rope_a(acc, which):
                    # a = q*ss (bf16), dst-slice = q*cs ; releases acc
                    a_t = []
                    for h in range(HPG):
                        a = atile.tile([128, 512], BF,
                                       name=f"a_{which}{h}_{lc}", tag="a")
                        nc.vector.tensor_mul(out=a, in0=acc[h], in1=ss_lc)
                        dst = (q_sb if which == "q" else k_sb)[h]
                        nc.vector.tensor_mul(
                            out=dst[:, ls_lo:ls_lo + 512], in0=acc[h], in1=cs_lc)
                        a_t.append(a)
                    return a_t

                def rope_b(a_t, which):
                    # dst -= quadrant_swap(a)   (all on DVE)
                    for h in range(HPG):
                        ash = atile.tile([128, 512], BF,
                                         name=f"ash_{which}{h}_{lc}", tag="a")
                        nc.vector.stream_shuffle(out=ash, in_=a_t[h],
                                                 mask=SWAP_MASK)
                        dst = (q_sb if which == "q" else k_sb)[h]
                        sl = dst[:, ls_lo:ls_lo + 512]
                        nc.vector.tensor_sub(out=sl, in0=sl, in1=ash)

                def v_pass():
                    # v pass (x tiles stationary -> natural [l, d] layout);
                    # e-outer so each wv chunk is consumed once, in its DMA
                    # arrival order
                    accv = [acc_tile(f"pv_{lc}_{ls}") for ls in range(4)]
                    for e in range(NE):
                        for ls in range(4):
                            nc.tensor.matmul(
                                accv[ls],
                                lhsT=xsl(e, ls * 128, (ls + 1) * 128),
                                rhs=wv_sb[:, e, :],
                                start=(e == 0), stop=(e == NE - 1))
                    for ls in range(4):
                        if ls % 2 == 0:
                            nc.scalar.copy(out=v_sb[:, lc * 4 + ls, :],
                                           in_=accv[ls])
                        else:
                            nc.vector.tensor_copy(out=v_sb[:, lc * 4 + ls, :],
                                                  in_=accv[ls])

                if lc == 0:
                    acc8 = qk_pass((0, 1))
                    accq, acck = acc8[:HPG], acc8[HPG:]
                    aq = rope_a(accq, "q")
                    ak = rope_a(acck, "k")
                    rope_b(aq, "q")
                    v_pass()
                    rope_b(ak, "k")
                elif lc == NLC - 1:
                    # last chunk: finalize k before the v pass so attention's
                    # first score matmuls aren't gated on the v matmuls
                    accq = qk_pass((0,))
                    aq = rope_a(accq, "q")
                    acck = qk_pass((1,))
                    rope_b(aq, "q")
                    ak = rope_a(acck, "k")
                    rope_b(ak, "k")
                    v_pass()
                else:
                    accq = qk_pass((0,))
                    aq = rope_a(accq, "q")
                    acck = qk_pass((1,))
                    rope_b(aq, "q")
                    ak = rope_a(acck, "k")
                    v_pass()
                    rope_b(ak, "k")

        # ======== phase 2+3: causal attention with interleaved projection ==
        # jobs are ic-major: once all 4 heads finished l-chunk ic, that
        # chunk's output projection is emitted immediately — it fills
        # attention pipeline bubbles and spreads the output DMA.
        nc.gpsimd.dma_start(
            out=masks_sb, in_=d["masks"].rearrange("p (r f) -> p r f", r=4))
        nc.gpsimd.dma_start(out=ones_sb, in_=d["ones"])
        with tc.tile_pool(name="pexp", bufs=10) as pexp, \
             tc.tile_pool(name="zacc", bufs=3) as zacc, \
             tc.tile_pool(name="zpool", bufs=3) as zpool, \
             tc.tile_pool(name="outst", bufs=3) as outst, \
             tc.tile_pool(name="psum2", bufs=1, space="PSUM") as ps2:
            jobs = [(h, ic) for ic in range(NLC) for h in range(HPG)]
            # per-job block order: diagonal blocks first (their mask-muls
            # land while the za chain is short), then the full blocks -- the
            # job tail is then pure adds and the DVE catches up before the
            # denominator matmuls.  Blocks are processed in PAIRS sharing one
            # [128, 1024] psum tile, halving the scalar engine's
            # per-activation overhead (exp throughput paces the big jobs).
            jseq = {ji: list(range(4 * ic, 4 * ic + 4)) + list(range(4 * ic))
                    for ji, (_h, ic) in enumerate(jobs)}
            steps = [(ji, pi)
                     for ji in range(len(jobs))
                     for pi in range(len(jseq[ji]) // 2)]
            LA = 1
            pss_map = {}
            psy_map = {}
            za_map = {}
            psz_map = {}
            # output-projection work queue: quarters (lcx, lt, fq) are
            # interleaved one-per-step into the FOLLOWING chunk's attention
            # jobs -- the projection is scalar-light and PE-heavy, which pads
            # each window so the scalar engine's exp stream keeps up
            pending_proj = []
            ot_map = {}
            proj_stride = [1]
            proj_tick = [0]

            def emit_proj_quarter(tag="po"):
                lcx, lt, fq = pending_proj.pop(0)
                l0 = lcx * 512 + lt * 128
                if fq == 0:
                    ot_map[(lcx, lt)] = outst.tile(
                        [128, E], mybir.dt.float16,
                        name=f"ot_{lcx}_{lt}", tag="ot")
                ot = ot_map[(lcx, lt)]
                po = ps2.tile([128, 512], FP32, name=f"po_{lcx}_{lt}_{fq}",
                              tag=tag, bufs=1)
                for hh in range(HPG):
                    nc.tensor.matmul(
                        po,
                        lhsT=y_sb[hh][:, l0:l0 + 128],
                        rhs=wout_sb[:, hh, fq * 512:(fq + 1) * 512],
                        start=(hh == 0), stop=(hh == HPG - 1))
                if fq < 3:
                    nc.vector.tensor_copy(
                        out=ot[:, fq * 512:(fq + 1) * 512], in_=po)
                else:
                    nc.scalar.copy(
                        out=ot[:, fq * 512:(fq + 1) * 512], in_=po)
                    eng = (nc.sync, nc.gpsimd, nc.scalar, nc.sync)[lt % 4]
                    eng.dma_start(out=d["out"][l0:l0 + 128, :],
                                  in_=ot_map.pop((lcx, lt)))

            def blk(ji, pos):
                _h, ic = jobs[ji]
                jb = jseq[ji][pos]
                # diagonal blocks (r >= 1) have no valid columns below
                # f = 128*r: compute only the valid column range
                r = jb - 4 * ic
                return jb, r, (r * 128 if r > 0 else 0)

            def emit_s(ji, pi):
                h, ic = jobs[ji]
                t = ps2.tile([128, 1024], FP32, name=f"pss_{ji}_{pi}",
                             tag="pss", bufs=2)
                for k in (0, 1):
                    jb, r, lo = blk(ji, 2 * pi + k)
                    nc.tensor.matmul(
                        t[:, 512 * k + lo:512 * (k + 1)],
                        lhsT=k_sb[h][:, jb * 128:(jb + 1) * 128],
                        rhs=q_sb[h][:, ic * 512 + lo:(ic + 1) * 512],
                        start=True, stop=True)
                pss_map[(ji, pi)] = t

            def finalize(ji):
                # denominator + normalization for job ji, emitted early in
                # job ji+1: the za chain gets a full job of slack, so the PE
                # never waits on it, and Z is a single 512-col matmul
                h, ic = jobs[ji]
                psz = ps2.tile([128, 512], FP32, name=f"psz_{ji}",
                               tag="psz", bufs=1)
                nc.tensor.matmul(psz, lhsT=ones_sb, rhs=za_map.pop(ji),
                                 start=True, stop=True)
                zv = zpool.tile([128, 512], FP32, name=f"zinv_{ji}",
                                tag="zinv")
                nc.vector.reciprocal_approx_fast(out=zv, in_=psz)
                nc.vector.tensor_mul(
                    out=y_sb[h][:, ic * 512:(ic + 1) * 512],
                    in0=psy_map.pop(ji), in1=zv)
                if h == HPG - 1:
                    # all heads done for this l-chunk: queue its output
                    # projection ([l, f] orientation, full-E rows so the
                    # store DMA moves 4 KB per partition line); quarters
                    # are spread over the following steps
                    pending_proj.extend(
                        (ic, lt, fq) for lt in range(4) for fq in range(4))
                    if ic + 1 < NLC:
                        nsteps = HPG * (4 * (ic + 1) + 4) // 2
                        proj_stride[0] = max(1, nsteps // 16)
                    else:
                        proj_stride[0] = 1
                    proj_tick[0] = -1

            ptr = 0
            for idx, (ji, pi) in enumerate(steps):
                while ptr < len(steps) and ptr <= idx + LA:
                    emit_s(*steps[ptr])
                    ptr += 1
                h, ic = jobs[ji]
                npairs = (4 * ic + 4) // 2
                if pi == 1 and ji > 0:
                    finalize(ji - 1)
                if ji == 1 and pi == 0:
                    # wout is first needed by the ic=0 projection (~30us into
                    # the attention phase): deferring its load here keeps it
                    # clear of the startup bandwidth crunch
                    nc.gpsimd.dma_start(
                        out=wout_sb,
                        in_=d["wout"].rearrange("(h p) f -> p h f", p=128))
                if pi == 0:
                    psy_map[ji] = ps2.tile([128, 512], FP32, name=f"psy_{ji}",
                                           tag="psy", bufs=2)
                psy = psy_map[ji]
                pss = pss_map.pop((ji, pi))
                jbA, rA, loA = blk(ji, 2 * pi)
                jbB, rB, loB = blk(ji, 2 * pi + 1)
                pt = pexp.tile([128, 1024], FP16, name=f"pt_{ji}_{pi}",
                               tag="pexp", bufs=5)
                # one exp covers both halves (the dead zone [512:512+loB) of
                # a diagonal pair holds exp(stale psum) -- never read)
                nc.scalar.activation(out=pt[:, loA:], in_=pss[:, loA:],
                                     func=EXP, bias=ebias_sb)
                for k, r, lo in ((0, rA, loA), (1, rB, loB)):
                    if r >= 0:
                        # diagonal block: only the first 128 columns of the
                        # valid range hold the per-element triangle; the rest
                        # are all-1.  On gpsimd -- the DVE runs the za chain.
                        nc.gpsimd.tensor_mul(
                            out=pt[:, 512 * k + lo:512 * k + lo + 128],
                            in0=pt[:, 512 * k + lo:512 * k + lo + 128],
                            in1=masks_sb[:, r, lo:lo + 128])
                # running probability sum for the softmax denominator (DVE);
                # consumed by finalize() a full job later
                if pi == 0:
                    za = zacc.tile([128, 512], FP16, name=f"za_{ji}", tag="za")
                    za_map[ji] = za
                    nc.vector.tensor_copy(out=za, in_=pt[:, 0:512])
                    nc.vector.tensor_add(out=za[:, loB:], in0=za[:, loB:],
                                         in1=pt[:, 512 + loB:])
                else:
                    za = za_map[ji]
                    nc.vector.tensor_add(out=za[:, loA:], in0=za[:, loA:],
                                         in1=pt[:, loA:512])
                    nc.vector.tensor_add(out=za[:, loB:], in0=za[:, loB:],
                                         in1=pt[:, 512 + loB:])
                nc.tensor.matmul(psy[:, loA:],
                                 lhsT=v_sb[:, jbA, h * 128:(h + 1) * 128],
                                 rhs=pt[:, loA:512],
                                 start=(pi == 0), stop=False)
                nc.tensor.matmul(psy[:, loB:],
                                 lhsT=v_sb[:, jbB, h * 128:(h + 1) * 128],
                                 rhs=pt[:, 512 + loB:],
                                 start=False, stop=(pi == npairs - 1))
                if pending_proj:
                    proj_tick[0] += 1
                    if proj_tick[0] > 0 and proj_tick[0] % proj_stride[0] == 0:
                        emit_proj_quarter()

            # finalize the last job, then drain its projection, alternating
            # psum tags (the psz tag is free after the last reciprocal) so
            # back-to-back quarters don't serialize on one bank
            finalize(len(jobs) - 1)
            qn = 0
            while pending_proj:
                emit_proj_quarter("po" if qn % 2 == 0 else "psz")
                qn += 1


# ------------------------------------------------------------------ host side

# head-dim permutation: quadrant q holds rope pairs 16q..16q+15 as
# (even dims | odd dims), so the rope partner swap stays within a
# 32-partition quadrant (STREAM_SHUFFLE's reach)
_PERM_IDX = np.concatenate(
    [np.concatenate([np.arange(16) * 2 + 32 * q,
                     np.arange(16) * 2 + 1 + 32 * q])
     for q in range(4)])


def prep_in_maps(x, rope, w_attn, w_proj):
    x = np.asarray(x, np.float32)
    rope = np.asarray(rope, np.float32)
    w_attn = np.asarray(w_attn, np.float32)
    w_proj = np.asarray(w_proj, np.float32)

    sin = rope[:, :, 0].T                    # [64, L]
    cos = rope[:, :, 1].T
    cs = np.zeros((128, L), np.float32)
    ss = np.zeros((128, L), np.float32)
    for q in range(4):
        pr = slice(16 * q, 16 * (q + 1))     # pair indices of quadrant q
        cs[32 * q:32 * q + 16] = cos[pr]
        cs[32 * q + 16:32 * q + 32] = cos[pr]
        ss[32 * q:32 * q + 16] = -sin[pr]
        ss[32 * q + 16:32 * q + 32] = sin[pr]
    cs = (cs * SCALE).astype(BF16)
    ss = (ss * SCALE).astype(BF16)

    p = np.arange(128)[:, None]
    f = np.arange(512)[None, :]
    masks = np.zeros((128, 4, 512), np.float32)
    for r in range(4):
        masks[:, r, :] = (r * 128 + p <= f).astype(np.float32)
    masks = masks.reshape(128, 4 * 512).astype(np.float16)

    ones = np.ones((128, 128), np.float16)

    xt_b = [np.ascontiguousarray(x[b].T).astype(BF16) for b in range(B)]

    wqk_g, wv_g, wout_g = {}, {}, {}
    for g in range(G):
        heads = [g * HPG + hl for hl in range(HPG)]
        wq = [np.ascontiguousarray(
                 w_attn[h * 128:(h + 1) * 128, :][_PERM_IDX, :].T) for h in heads]
        wk = [np.ascontiguousarray(
                 w_attn[E + h * 128:E + (h + 1) * 128, :][_PERM_IDX, :].T)
              for h in heads]
        wqk_g[g] = np.concatenate(wq + wk, axis=1).astype(BF16)        # [E, 1024]
        wv_g[g] = np.concatenate(
            [w_attn[2 * E + h * 128:2 * E + (h + 1) * 128, :].T for h in heads],
            axis=1).astype(BF16)                                        # [E, 512]
        wout_g[g] = np.ascontiguousarray(
            w_proj[:, g * 512:(g + 1) * 512].T).astype(BF16)            # [512, E]

    in_maps = []
    for c in range(NCORES):
        b, g = divmod(c, G)
        in_maps.append({
            "xt": xt_b[b],
            "wqk": wqk_g[g],
            "wv": wv_g[g],
            "wout": wout_g[g],
            "cs": cs,
            "ss": ss,
            "masks": masks,
            "ones": ones,
        })
    return in_maps


def assemble_output(results):
    out = np.zeros((B, L, E), np.float32)
    for c in range(NCORES):
        b, g = divmod(c, G)
        out[b] += results[c]["out"]
    return out


_NC = None


def get_nc():
    global _NC
    if _NC is None:
        _NC = build_nc()
    return _NC


def run(x, rope, w_attn, w_proj, trace=False, tmpdir=None):
    nc = get_nc()
    in_maps = prep_in_maps(x, rope, w_attn, w_proj)
    kwargs = {}
    if trace:
        import sys
        import types
        from concourse import bass_utils as _bu
        try:
            from trn_agent_boot.trn_boot import _ntff_profile_via_ctypes
            hook = _ntff_profile_via_ctypes("/opt/axon/libaxon_pjrt.so")
            mod = types.ModuleType("antenv.axon_hooks")
            mod.get_axon_ntff_profile_hook = lambda: hook
            sys.modules["antenv.axon_hooks"] = mod
            _bu.upload_artifacts = lambda dd: dd
        except Exception as e:  # pragma: no cover
            print("trace hook unavailable:", e)
        kwargs = dict(trace=True, tmpdir=tmpdir)
    res = run_bass_kernel_spmd(nc, in_maps, core_ids=list(range(NCORES)), **kwargs)
    return assemble_output(res.results), res


def kernel(x, rope, w_attn, w_proj):
    out, _ = run(x, rope, w_attn, w_proj, trace=False)
    return out
